# revision 1
# baseline (speedup 1.0000x reference)
"""Trainium2 Bass kernel for LlamaAttention (B=2, S=2048, D=2048, H=16, HD=128).

Sharding: tensor-parallel over heads. Each of the 8 cores computes 2 heads:
q/k/v projections for its 256-column slice of Wq/Wk/Wv, rope, causal-masked
softmax attention, AV, and a partial output projection with its 256-row slice
of Wo. The 8 partial [B*S, D] outputs are summed on the host.

Device-side layout is feature-major ("transposed"): the host passes hs^T
[D, B*S] so every matmul contraction dim lands on SBUF partitions, and
attention runs on scores^T = k^T.T-blocks @ q^T tiles. Softmax is computed
without max subtraction (the host shifts the additive mask per-row and takes
exp of it, so P = exp(scale*S) * expmask, row sums via a PE ones-matmul, and
normalization is a reciprocal broadcast folded into the PSUM eviction).
"""

import os
import sys
from contextlib import ExitStack

import numpy as np

for _p in ("/opt/trn_rl_repo",):
    if _p not in sys.path:
        sys.path.insert(0, _p)

import ml_dtypes  # noqa: E402

import concourse.bass as bass  # noqa: E402
import concourse.tile as tile  # noqa: E402
from concourse import bacc, mybir  # noqa: E402
from concourse.masks import make_identity  # noqa: E402

B, S, D, H, HD = 2, 2048, 2048, 16, 128
T = B * S                    # 4096 tokens total
NCORES = 8
HPC = H // NCORES            # 2 heads per core
JC = HPC * HD                # 256 per-core feature columns
P = 128
TB = 512                     # token block for projections
NTB = T // TB                # 8
KT = D // P                  # 16 contraction tiles of 128
TQB = 512                    # tq block in attention
NTQB = S // TQB              # 4 per batch
NTK = S // P                 # 16 tk tiles per batch
SCALE = 1.0 / float(np.sqrt(HD))
ROPE_THETA = 10000.0

F32 = mybir.dt.float32
F32R = mybir.dt.float32r
F16 = mybir.dt.float16

# tile classes
CLS_SKIP, CLS_ZERO, CLS_MIXED = 0, 1, 2

_prog_cache: dict[bytes, tuple] = {}


def _r(ap):
    return ap.bitcast(F32R)


def _build_program(cls: np.ndarray):
    """Build the SPMD Bass program. cls: [B, NTK, NTQB] int8 tile classes
    (identical for every core — the mask does not depend on the head)."""
    has_mixed = bool((cls == CLS_MIXED).any())

    nc = bacc.Bacc(
        "TRN2",
        target_bir_lowering=False,
        debug=False,
        enable_asserts=True,
        num_devices=NCORES,
    )

    hsT_d = nc.dram_tensor("hsT", [D, T], F32R, kind="ExternalInput").ap()
    wq_d = nc.dram_tensor("wq", [P, KT * JC], F32R, kind="ExternalInput").ap()
    wk_d = nc.dram_tensor("wk", [P, KT * JC], F32R, kind="ExternalInput").ap()
    wv_d = nc.dram_tensor("wv", [P, KT * JC], F32R, kind="ExternalInput").ap()
    wo_d = nc.dram_tensor("wo", [P, HPC * D], F32R, kind="ExternalInput").ap()
    cos_d = nc.dram_tensor("cosT", [HD, T], F16, kind="ExternalInput").ap()
    sin_d = nc.dram_tensor("sinT", [HD, T], F16, kind="ExternalInput").ap()
    em_d = None
    if has_mixed:
        em_d = nc.dram_tensor("emT", [B, S, S], F32, kind="ExternalInput").ap()
    out_d = nc.dram_tensor("out", [T, D], F32, kind="ExternalOutput").ap()
    v_d = nc.dram_tensor("v_scratch", [B, HPC, P, NTK * P], F32R, kind="Internal").ap()

    hsT_v = hsT_d.rearrange("(kt p) t -> p kt t", p=P)       # [128, 16, 4096]

    with tile.TileContext(nc) as tc, ExitStack() as ctx:
        # ---------- long-lived tiles ----------
        persist = ctx.enter_context(tc.tile_pool(name="persist", bufs=1))
        qT = persist.tile([P, HPC * T], F32R)     # (j2, t) feature-major q
        kT = persist.tile([P, HPC * T], F32R)
        ones_f = persist.tile([P, 1], F32)
        ones_t = persist.tile([P, 1], F32R)
        ident = persist.tile([P, P], F32)
        nc.any.memset(ones_f[:], 1.0)
        nc.vector.tensor_copy(ones_t[:], ones_f[:])
        make_identity(nc, ident[:])

        qT_v = qT[:].rearrange("p (j2 t) -> p j2 t", j2=HPC)
        kT_v = kT[:].rearrange("p (j2 t) -> p j2 t", j2=HPC)

        # ---------- phase 1: q/k/v projections (+rope, v transpose) ----------
        with tc.tile_pool(name="wpool", bufs=1) as wpool, \
             tc.tile_pool(name="cspool", bufs=1) as cspool, \
             tc.tile_pool(name="hstp", bufs=3) as hstp, \
             tc.tile_pool(name="stg", bufs=2) as stg, \
             tc.tile_pool(name="vstg", bufs=4) as vstgp, \
             tc.tile_pool(name="ppsum", bufs=1, space="PSUM") as pps, \
             tc.tile_pool(name="vtpsum", bufs=2, space="PSUM") as vtp:
            wq_s = wpool.tile([P, KT * JC], F32R)
            wk_s = wpool.tile([P, KT * JC], F32R)
            wv_s = wpool.tile([P, KT * JC], F32R)
            for kt in range(KT):
                ksl = slice(kt * JC, (kt + 1) * JC)
                for w_s, w_d_ in ((wq_s, wq_d), (wk_s, wk_d), (wv_s, wv_d)):
                    nc.sync.dma_start(w_s[:, ksl], w_d_[:, ksl])
            cos_s = cspool.tile([HD, T], F16)
            sin_s = cspool.tile([HD, T], F16)
            nc.sync.dma_start(cos_s[:], cos_d)
            nc.sync.dma_start(sin_s[:], sin_d)

            w_views = [
                w[:].rearrange("p (kt j) -> p kt j", j=JC)
                for w in (wq_s, wk_s, wv_s)
            ]

            KTH = KT // 2  # k-tiles per half-load of hs^T
            for tb in range(NTB):
                halves = []
                for hf in range(2):
                    hst = hstp.tile([P, KTH * TB], F32R, tag="hst")
                    for kl in range(KTH):
                        nc.sync.dma_start(
                            hst[:, kl * TB:(kl + 1) * TB],
                            hsT_v[:, hf * KTH + kl,
                                  tb * TB:(tb + 1) * TB],
                        )
                    halves.append(
                        hst[:].rearrange("p (kt t) -> p kt t", t=TB))
                for pi in range(3):
                    for j2 in range(HPC):
                        ps = pps.tile([P, TB], F32, tag=f"pp{pi}{j2}")
                        for kt in range(KT):
                            nc.tensor.matmul(
                                ps[:],
                                lhsT=w_views[pi][:, kt, j2 * P:(j2 + 1) * P],
                                rhs=halves[kt // KTH][:, kt % KTH, :],
                                start=(kt == 0),
                                stop=(kt == KT - 1),
                            )
                        tsl = slice(tb * TB, (tb + 1) * TB)
                        if pi < 2:
                            # rope folded into the PSUM eviction:
                            # out[:64]  = x1*cos - x2*sin ; out[64:] = x2*cos + x1*sin
                            dst = (qT_v if pi == 0 else kT_v)[:, j2, tsl]
                            c1 = stg.tile([P, TB], F32, tag="ropeA")
                            c2 = stg.tile([P, TB], F32, tag="ropeB")
                            nc.vector.tensor_mul(c1[:], ps[:], cos_s[:, tsl])
                            # cross products (operands at different base partitions)
                            nc.vector.tensor_mul(
                                c2[0:64, :], ps[64:128, :], sin_s[0:64, tsl])
                            nc.vector.tensor_mul(
                                c2[64:128, :], ps[0:64, :], sin_s[64:128, tsl])
                            nc.vector.tensor_sub(
                                dst[0:64, :], c1[0:64, :], c2[0:64, :])
                            nc.vector.tensor_add(
                                dst[64:128, :], c1[64:128, :], c2[64:128, :])
                        else:
                            # v: evict, transpose 128x128 blocks on PE, spill to DRAM
                            vs = stg.tile([P, TB], F32, tag="vstage")
                            nc.scalar.copy(vs[:], ps[:])
                            vo = vstgp.tile([P, 4 * P], F32R, tag="vo")
                            for k in range(TB // P):
                                vps = vtp.tile([P, P], F32, tag="vt")
                                nc.tensor.matmul(
                                    vps[:],
                                    lhsT=vs[:, k * P:(k + 1) * P],
                                    rhs=ident[:],
                                    is_transpose=True,
                                )
                                nc.scalar.copy(
                                    vo[:, k * P:(k + 1) * P], vps[:])
                            b_idx = (tb * TB) // S
                            tk0 = ((tb * TB) % S) // P
                            nc.sync.dma_start(
                                v_d[b_idx, j2, :,
                                    tk0 * P:(tk0 + 4) * P],
                                vo[:],
                            )

        # ---------- phase 2+3: attention, then per-batch output projection ----------
        persist2 = ctx.enter_context(tc.tile_pool(name="persist2", bufs=1))
        aT = persist2.tile([P, HPC * T], F32R)    # attn out^T (j2, t)
        aT_v = aT[:].rearrange("p (j2 t) -> p j2 t", j2=HPC)
        wo_s = persist2.tile([P, HPC * D], F32R)
        nc.sync.dma_start(wo_s[:], wo_d)
        wo_sv = wo_s[:].rearrange("p (j2 n) -> p j2 n", j2=HPC)

        with tc.tile_pool(name="vbp", bufs=2) as vbp, \
             tc.tile_pool(name="ptp", bufs=8) as ptp, \
             tc.tile_pool(name="emp", bufs=4) as emp, \
             tc.tile_pool(name="smp", bufs=4) as smp, \
             tc.tile_pool(name="ostg", bufs=4) as ostgp, \
             tc.tile_pool(name="spsum", bufs=2, space="PSUM") as sps, \
             tc.tile_pool(name="opsum", bufs=1, space="PSUM") as ops, \
             tc.tile_pool(name="supsum", bufs=1, space="PSUM") as sups:
            for b in range(B):
                vb_vs = []
                for h in range(HPC):
                    vb = vbp.tile([P, NTK * P], F32R, tag=f"vb{h}")
                    nc.sync.dma_start(vb[:], v_d[b, h])
                    vb_vs.append(vb[:].rearrange("p (tk j) -> p tk j", j=P))
                for tqb in range(NTQB):
                    tq0 = b * S + tqb * TQB
                    live = [tk for tk in range(NTK)
                            if cls[b, tk, tqb] != CLS_SKIP]
                    o_ps = [ops.tile([P, TQB], F32, tag=f"ops{h}{tqb % 2}",
                                     name=f"o_ps{h}") for h in range(HPC)]
                    s_ps = [sups.tile([1, TQB], F32, tag=f"sum{h}",
                                      name=f"s_ps{h}") for h in range(HPC)]
                    for i, tk in enumerate(live):
                        em = None
                        if cls[b, tk, tqb] == CLS_MIXED:
                            em = emp.tile([P, TQB], F32, tag="em")
                            nc.sync.dma_start(
                                em[:],
                                em_d[b, tk * P:(tk + 1) * P,
                                     tqb * TQB:(tqb + 1) * TQB],
                            )
                        for h in range(HPC):
                            st = sps.tile([P, TQB], F32, tag="st")
                            nc.tensor.matmul(
                                st[:],
                                lhsT=kT_v[:, h, b * S + tk * P:
                                          b * S + (tk + 1) * P],
                                rhs=qT_v[:, h, tq0:tq0 + TQB],
                                start=True, stop=True,
                            )
                            pt = ptp.tile([P, TQB], F32R, tag="pt")
                            nc.scalar.activation(
                                pt[:], st[:],
                                mybir.ActivationFunctionType.Exp,
                                scale=SCALE,
                            )
                            if em is not None:
                                nc.vector.tensor_mul(pt[:], pt[:], em[:])
                            nc.tensor.matmul(
                                o_ps[h][:],
                                lhsT=vb_vs[h][:, tk, :],
                                rhs=pt[:],
                                start=(i == 0), stop=(i == len(live) - 1),
                            )
                            nc.tensor.matmul(
                                s_ps[h][:],
                                lhsT=ones_t[:],
                                rhs=pt[:],
                                start=(i == 0), stop=(i == len(live) - 1),
                            )
                    for h in range(HPC):
                        rc = smp.tile([1, TQB], F32, tag="rc")
                        nc.vector.reciprocal(rc[:], s_ps[h][:])
                        rbc = smp.tile([P, TQB], F32, tag="rbc")
                        nc.gpsimd.partition_broadcast(rbc[:], rc[:])
                        nc.vector.tensor_mul(
                            aT_v[:, h, tq0:tq0 + TQB], o_ps[h][:], rbc[:])
                # output projection for this batch's tokens
                for tb32 in range(b * S // P, (b + 1) * S // P):
                    for nb in range(D // 512):
                        ps = ops.tile([P, 512], F32, tag=f"ops{0 if nb % 2 == 0 else 1}{tb32 % 2}", name="ps")
                        for j2 in range(HPC):
                            nc.tensor.matmul(
                                ps[:],
                                lhsT=aT_v[:, j2, tb32 * P:(tb32 + 1) * P],
                                rhs=wo_sv[:, j2, nb * 512:(nb + 1) * 512],
                                start=(j2 == 0), stop=(j2 == HPC - 1),
                            )
                        og = ostgp.tile([P, 512], F32, tag="og")
                        nc.vector.tensor_copy(og[:], ps[:])
                        nc.sync.dma_start(
                            out_d[tb32 * P:(tb32 + 1) * P,
                                  nb * 512:(nb + 1) * 512],
                            og[:],
                        )

    nc.compile()
    return nc


def _host_prep(hidden_states, attention_mask, position_ids):
    hs2 = np.ascontiguousarray(
        hidden_states.reshape(T, D).astype(np.float32, copy=False))
    hsT = np.ascontiguousarray(hs2.T)                       # [D, T]

    # rope tables gathered by position_ids, feature-major, sign baked into sin
    inv_freq = 1.0 / (ROPE_THETA ** (np.arange(0, HD, 2, dtype=np.float32) / HD))
    pos = np.asarray(position_ids).astype(np.int64)
    maxpos = int(pos.max()) + 1
    t_ar = np.arange(maxpos, dtype=np.float32)
    freqs = np.outer(t_ar, inv_freq)                        # [maxpos, 64]
    emb = np.concatenate([freqs, freqs], axis=-1)           # [maxpos, 128]
    cos_tab = np.cos(emb).astype(np.float32)
    sin_tab = np.sin(emb).astype(np.float32)
    cos_g = cos_tab[pos]                                    # [B, S, HD]
    sin_g = sin_tab[pos]
    cosT = np.concatenate([cos_g[b].T for b in range(B)], axis=1)  # [HD, T]
    sinT = np.concatenate([sin_g[b].T for b in range(B)], axis=1)
    cosT = cosT.astype(np.float16)
    sinT = sinT.astype(np.float16)

    # shifted-exp mask, transposed per batch, plus tile classification
    m = np.asarray(attention_mask, dtype=np.float32)[:, 0]  # [B, S(tq), S(tk)]
    rowmax = m.max(axis=-1, keepdims=True)
    em = np.exp(m - rowmax)                                 # [B, tq, tk] in [0,1]
    emT = np.ascontiguousarray(em.transpose(0, 2, 1))       # [B, tk, tq]
    emr = emT.reshape(B, NTK, P, NTQB, TQB)
    tmax = emr.max(axis=(2, 4))                             # [B, NTK, NTQB]
    tmin = emr.min(axis=(2, 4))
    cls = np.full((B, NTK, NTQB), CLS_MIXED, dtype=np.int8)
    cls[tmax == 0.0] = CLS_SKIP
    cls[(tmin == 1.0) & (tmax == 1.0)] = CLS_ZERO
    # guard: a fully-skipped tq column would leave PSUM unwritten
    for b in range(B):
        for tqb in range(NTQB):
            if (cls[b, :, tqb] == CLS_SKIP).all():
                cls[b, 0, tqb] = CLS_MIXED
    return hsT, cosT, sinT, emT, cls


def kernel(hidden_states, attention_mask, position_ids, Wq, Wk, Wv, Wo):
    hsT, cosT, sinT, emT, cls = _host_prep(
        hidden_states, attention_mask, position_ids)

    key = cls.tobytes()
    if key not in _prog_cache:
        _prog_cache[key] = _build_program(cls)
    nc = _prog_cache[key]
    has_mixed = bool((cls == CLS_MIXED).any())

    Wq = np.asarray(Wq, dtype=np.float32)
    Wk = np.asarray(Wk, dtype=np.float32)
    Wv = np.asarray(Wv, dtype=np.float32)
    Wo = np.asarray(Wo, dtype=np.float32)

    in_maps = []
    for c in range(NCORES):
        jsl = slice(c * JC, (c + 1) * JC)
        def sb_w(w):  # [D, JC] -> SBUF layout [128, KT*JC]
            return np.ascontiguousarray(
                w.reshape(KT, P, JC).transpose(1, 0, 2).reshape(P, KT * JC))

        m = {
            "hsT": hsT,
            "wq": sb_w(Wq[:, jsl]),
            "wk": sb_w(Wk[:, jsl]),
            "wv": sb_w(Wv[:, jsl]),
            "wo": np.ascontiguousarray(
                Wo[jsl, :].reshape(HPC, P, D).transpose(1, 0, 2).reshape(P, HPC * D)),
            "cosT": cosT,
            "sinT": sinT,
        }
        if has_mixed:
            m["emT"] = emT
        in_maps.append(m)

    if os.environ.get("KERNEL_SIM"):
        from concourse.bass_interp import CoreSim
        outs = []
        for c in range(int(os.environ.get("KERNEL_SIM_CORES", "1"))):
            sim = CoreSim(nc, require_finite=False, require_nnan=True)
            for k, v in in_maps[c].items():
                sim.tensor(k)[:] = v
            sim.simulate(check_with_hw=False)
            outs.append(np.array(sim.tensor("out")))
        total = np.sum(np.stack(outs, 0), axis=0)
        return total.reshape(B, S, D).astype(np.float32)

    from concourse.bass_utils import run_bass_kernel_spmd
    trace = bool(os.environ.get("KERNEL_TRACE"))
    res = run_bass_kernel_spmd(
        nc, in_maps, core_ids=list(range(NCORES)), trace=trace)
    if trace and res.exec_time_ns is not None:
        print(f"HW exec time: {res.exec_time_ns} ns")
        kernel.last_exec_time_ns = res.exec_time_ns
        kernel.last_trace = res.instructions_and_trace
    partials = np.stack([r["out"] for r in res.results], axis=0)
    total = partials.sum(axis=0)
    return total.reshape(B, S, D).astype(np.float32)



# revision 21
# speedup vs baseline: 1.2475x; 1.2475x over previous
"""Trainium2 Bass kernel for LlamaAttention (B=2, S=2048, D=2048, H=16, HD=128).

Sharding: batch-split x head tensor-parallel. Cores 0-3 take batch 0, cores
4-7 batch 1; within a group each core owns 4 heads (512 feature columns of
Wq/Wk/Wv, 512 rows of Wo). Each core computes q/k/v projections + rope for
its heads, causal-masked softmax attention, AV, and a partial output
projection; the host sums the 4 partials per batch.

All matmul operands are fp16 (PSUM accumulation stays fp32; the softmax
denominator pipeline is fp32). Device layout is feature-major: hs^T [D, S]
so contraction dims land on SBUF partitions; attention runs on
scores^T = k-block^T @ q^T tiles. Softmax uses the host-shifted mask trick
(P = exp(scale*S) * em with em in [0,1]); row sums via a PE ones-matmul
into partition-sliced rows of one PSUM bank; normalization via
reciprocal_approx_fast + gpsimd broadcast folded into the PSUM eviction.
V never spills to DRAM: it is PE-transposed and kept resident in SBUF.
The unique causal-boundary mask tiles (4 patterns) are loaded once and
kept resident. Output projection for token block i is emitted while
attention for block i+1 runs, keeping the PE queue dense.
"""

import os
import sys
from contextlib import ExitStack

import numpy as np

for _p in ("/opt/trn_rl_repo",):
    if _p not in sys.path:
        sys.path.insert(0, _p)

import ml_dtypes  # noqa: E402

import concourse.bass as bass  # noqa: E402,F401
import concourse.tile as tile  # noqa: E402
from concourse import bacc, mybir  # noqa: E402
from concourse.masks import make_identity  # noqa: E402

B, S, D, H, HD = 2, 2048, 2048, 16, 128
NCORES = 8
CPG = 4                      # cores per batch group
HPC = H // CPG               # 4 heads per core
JC = HPC * HD                # 512 per-core feature columns
P = 128
TB = 512                     # token block for projections
NTB = S // TB                # 4 per core (one batch)
KT = D // P                  # 16 contraction tiles
TQB = 512                    # tq block in attention
NTQB = S // TQB              # 4
NTK = S // P                 # 16 tk tiles
NBW = 256                    # output-projection free-dim block
SCALE = 1.0 / float(np.sqrt(HD))
ROPE_THETA = 10000.0

F32 = mybir.dt.float32
F16 = mybir.dt.float16

# tile classes
CLS_SKIP, CLS_ZERO, CLS_MIXED = 0, 1, 2

_prog_cache: dict[tuple, object] = {}


def _build_program(cls: np.ndarray, uidseq: tuple, n_uniq: int):
    """cls: [NTK, NTQB] int8 tile classes (shared by both batches).
    uidseq: for each mixed tile in (tqb, tk) scan order, the index of its
    mask pattern inside the resident em tensor."""
    nc = bacc.Bacc(
        "TRN2",
        target_bir_lowering=False,
        debug=False,
        enable_asserts=True,
        num_devices=NCORES,
    )

    hsT_d = nc.dram_tensor("hsT", [D, S], F16, kind="ExternalInput").ap()
    wq_d = nc.dram_tensor("wq", [P, KT * JC], F16, kind="ExternalInput").ap()
    wk_d = nc.dram_tensor("wk", [P, KT * JC], F16, kind="ExternalInput").ap()
    wv_d = nc.dram_tensor("wv", [P, KT * JC], F16, kind="ExternalInput").ap()
    wo_d = nc.dram_tensor("wo", [P, HPC * D], F16, kind="ExternalInput").ap()
    cos_d = nc.dram_tensor("cosT", [HD, S], F16, kind="ExternalInput").ap()
    sin_d = nc.dram_tensor("sinT", [HD, S], F16, kind="ExternalInput").ap()
    em_d = nc.dram_tensor("emU", [P, max(n_uniq, 1) * TQB], F16,
                          kind="ExternalInput").ap()
    out_d = nc.dram_tensor("out", [S, D], F16, kind="ExternalOutput").ap()

    hsT_v = hsT_d.rearrange("(kt p) t -> p kt t", p=P)       # [128, 16, 2048]

    # mixed-tile uid lookup in (tqb, tk) scan order
    uid_of = {}
    ui = 0
    for tqb in range(NTQB):
        for tk in range(NTK):
            if cls[tk, tqb] == CLS_MIXED:
                uid_of[(tk, tqb)] = uidseq[ui]
                ui += 1
    assert ui == len(uidseq)

    with tile.TileContext(nc) as tc, ExitStack() as ctx:
        # ---------- long-lived tiles ----------
        persist = ctx.enter_context(tc.tile_pool(name="persist", bufs=1))
        qT = persist.tile([P, HPC * S], F16)      # [hd, (h, t)]
        kT = persist.tile([P, HPC * S], F16)
        vT = persist.tile([P, HPC * NTK * P], F16)  # [tok%128, (h, tk, hd)]
        ident = persist.tile([P, P], F16)
        make_identity(nc, ident[:])
        # ones4[:, h, :]: column 32*h all-ones. Lands head h's exp-sum on
        # partition 32*h of a shared [97, TQB] PSUM accumulation region --
        # the only partition offsets engines may address are 0/32/64/96.
        ones4 = persist.tile([P, HPC * 97], F16)
        nc.any.memset(ones4[:], 0.0)
        ones4_v = ones4[:].rearrange("p (h c) -> p h c", h=HPC)
        for h in range(HPC):
            nc.any.memset(ones4_v[:, h, 32 * h:32 * h + 1], 1.0)

        qT_v = qT[:].rearrange("p (h t) -> p h t", h=HPC)
        kT_v = kT[:].rearrange("p (h t) -> p h t", h=HPC)
        vT_v = vT[:].rearrange("p (h m j) -> p h m j", h=HPC, m=NTK)

        # ---------- phase 1: q/k/v projections (+rope, v transpose) ----------
        with tc.tile_pool(name="wpool", bufs=1) as wpool, \
             tc.tile_pool(name="cspool", bufs=1) as cspool, \
             tc.tile_pool(name="hstp", bufs=2) as hstp, \
             tc.tile_pool(name="stg", bufs=2) as stg, \
             tc.tile_pool(name="vstg", bufs=2) as vstgp, \
             tc.tile_pool(name="ppsum", bufs=2, space="PSUM") as pps, \
             tc.tile_pool(name="vtpsum", bufs=2, space="PSUM") as vtp:
            wq_s = wpool.tile([P, KT * JC], F16)
            wk_s = wpool.tile([P, KT * JC], F16)
            wv_s = wpool.tile([P, KT * JC], F16)
            for w_s, w_dd in ((wq_s, wq_d), (wk_s, wk_d), (wv_s, wv_d)):
                nc.sync.dma_start(w_s[:], w_dd)
            cos_s = cspool.tile([HD, S], F16)
            sin_s = cspool.tile([HD, S], F16)
            nc.sync.dma_start(cos_s[:], cos_d)
            nc.sync.dma_start(sin_s[:], sin_d)

            w_views = [
                w[:].rearrange("p (kt j) -> p kt j", j=JC)
                for w in (wq_s, wk_s, wv_s)
            ]

            for tb in range(NTB):
                hst = hstp.tile([P, KT * TB], F16, tag="hst")
                for kt in range(KT):
                    nc.sync.dma_start(
                        hst[:, kt * TB:(kt + 1) * TB],
                        hsT_v[:, kt, tb * TB:(tb + 1) * TB],
                    )
                hst_v = hst[:].rearrange("p (kt t) -> p kt t", t=TB)
                tsl = slice(tb * TB, (tb + 1) * TB)
                for pi in range(3):
                    for j2 in range(HPC):
                        ps = pps.tile([P, TB], F32, tag="pp")
                        for kt in range(KT):
                            nc.tensor.matmul(
                                ps[:],
                                lhsT=w_views[pi][:, kt, j2 * P:(j2 + 1) * P],
                                rhs=hst_v[:, kt, :],
                                start=(kt == 0),
                                stop=(kt == KT - 1),
                            )
                        if pi < 2:
                            # rope folded into the PSUM eviction:
                            # out[:64] = x1*cos - x2*sin ; out[64:] = x2*cos + x1*sin
                            dst = (qT_v if pi == 0 else kT_v)[:, j2, tsl]
                            c1 = stg.tile([P, TB], F32, tag="ropeA")
                            c2 = stg.tile([P, TB], F32, tag="ropeB")
                            nc.vector.tensor_mul(c1[:], ps[:], cos_s[:, tsl])
                            nc.vector.tensor_mul(
                                c2[0:64, :], ps[64:128, :], sin_s[0:64, tsl])
                            nc.vector.tensor_mul(
                                c2[64:128, :], ps[0:64, :], sin_s[64:128, tsl])
                            nc.vector.tensor_sub(
                                dst[0:64, :], c1[0:64, :], c2[0:64, :])
                            nc.vector.tensor_add(
                                dst[64:128, :], c1[64:128, :], c2[64:128, :])
                        else:
                            # v: evict to fp16, transpose 128x128 blocks on PE
                            vs = vstgp.tile([P, TB], F16, tag="vstage")
                            nc.scalar.copy(vs[:], ps[:])
                            for k in range(TB // P):
                                vps = vtp.tile([P, P], F16, tag="vt")
                                nc.tensor.matmul(
                                    vps[:],
                                    lhsT=vs[:, k * P:(k + 1) * P],
                                    rhs=ident[:],
                                    is_transpose=True,
                                )
                                nc.scalar.copy(
                                    vT_v[:, j2, tb * (TB // P) + k, :], vps[:])

        # ---------- phase 2: attention with interleaved output projection ----
        persist2 = ctx.enter_context(tc.tile_pool(name="persist2", bufs=1))
        aT = persist2.tile([P, HPC * S], F16)     # attn out^T [hd, (h, t)]
        aT_v = aT[:].rearrange("p (h t) -> p h t", h=HPC)
        wo_s = persist2.tile([P, HPC * D], F16)
        nc.sync.dma_start(wo_s[:], wo_d)
        wo_sv = wo_s[:].rearrange("p (h n) -> p h n", h=HPC)
        em_s = persist2.tile([P, max(n_uniq, 1) * TQB], F16)
        nc.sync.dma_start(em_s[:], em_d)

        with tc.tile_pool(name="ptp", bufs=2 * HPC) as ptp, \
             tc.tile_pool(name="smp", bufs=2) as smp, \
             tc.tile_pool(name="rbcp", bufs=HPC) as rbcp, \
             tc.tile_pool(name="ostg", bufs=2) as ostgp, \
             tc.tile_pool(name="spsum", bufs=2, space="PSUM") as sps, \
             tc.tile_pool(name="opsum", bufs=1, space="PSUM") as ops, \
             tc.tile_pool(name="avpsum", bufs=1, space="PSUM") as avp, \
             tc.tile_pool(name="supsum", bufs=1, space="PSUM") as sups:
            o_ps = [avp.tile([P, TQB], F32, tag=f"av{h}", name=f"o_ps{h}")
                    for h in range(HPC)]
            # head h's sums live on partition 32*h of one PSUM bank
            sums = sups.tile([97, TQB], F32, tag="sums", name="sums")

            # one PSUM bank; the two 1KB halves ping-pong as accumulation
            # regions for successive output-projection groups
            op2 = ops.tile([P, 2 * NBW], F32, tag="op2", name="op2")
            dbg_rbcs: list = []
            dbg_extra: list = []

            def emit_oproj(t):
                # output projection for token block t (tokens t*TQB ..)
                for tb32 in range(t * (TQB // P), (t + 1) * (TQB // P)):
                    for nb in range(D // NBW):
                        pso = op2[:, (nb % 2) * NBW:(nb % 2 + 1) * NBW]
                        for j2 in range(HPC):
                            nc.tensor.matmul(
                                pso,
                                lhsT=aT_v[:, j2, tb32 * P:(tb32 + 1) * P],
                                rhs=wo_sv[:, j2, nb * NBW:(nb + 1) * NBW],
                                start=(j2 == 0), stop=(j2 == HPC - 1),
                            )
                        og = ostgp.tile([P, NBW], F16, tag="og")
                        nc.vector.tensor_copy(og[:], pso)
                        nc.sync.dma_start(
                            out_d[tb32 * P:(tb32 + 1) * P,
                                  nb * NBW:(nb + 1) * NBW],
                            og[:],
                        )

            for tqb in range(NTQB):
                tq0 = tqb * TQB
                live = [tk for tk in range(NTK) if cls[tk, tqb] != CLS_SKIP]
                L = len(live)
                pts = [[None] * HPC for _ in range(2)]
                for i, tk in enumerate(live):
                    for h in range(HPC):
                        st = sps.tile([P, TQB], F32, tag="st")
                        nc.tensor.matmul(
                            st[:],
                            lhsT=kT_v[:, h, tk * P:(tk + 1) * P],
                            rhs=qT_v[:, h, tq0:tq0 + TQB],
                            start=True, stop=True,
                        )
                        pt = ptp.tile([P, TQB], F16,
                                      tag="pt")
                        nc.scalar.activation(
                            pt[:], st[:],
                            mybir.ActivationFunctionType.Exp,
                            scale=SCALE,
                        )
                        if cls[tk, tqb] == CLS_MIXED:
                            # out-of-place: PE must never observe pre-mask pt
                            u = uid_of[(tk, tqb)]
                            ptm = ptmp.tile([P, TQB], F16, tag="ptm")
                            nc.vector.tensor_mul(
                                ptm[:], pt[:],
                                em_s[:, u * TQB:(u + 1) * TQB])
                            pt = ptm
                        pts[i % 2][h] = pt
                        if i > 0:
                            ptp_prev = pts[(i - 1) % 2][h]
                            nc.tensor.matmul(
                                o_ps[h][:],
                                lhsT=vT_v[:, h, live[i - 1], :],
                                rhs=ptp_prev[:],
                                start=(i - 1 == 0), stop=False,
                            )
                            nc.tensor.matmul(
                                sums[:],
                                lhsT=ones4_v[:, h, :],
                                rhs=ptp_prev[:],
                                start=(i - 1 == 0 and h == 0), stop=False,
                            )
                for h in range(HPC):
                    pt_last = pts[(L - 1) % 2][h]
                    nc.tensor.matmul(
                        o_ps[h][:],
                        lhsT=vT_v[:, h, live[L - 1], :],
                        rhs=pt_last[:],
                        start=(L == 1), stop=True,
                    )
                    nc.tensor.matmul(
                        sums[:],
                        lhsT=ones4_v[:, h, :],
                        rhs=pt_last[:],
                        start=(L == 1 and h == 0), stop=(h == HPC - 1),
                    )
                # eviction: per-head reciprocal (PSUM read is exempt from the
                # SBUF start-partition rule), broadcast, normalize into aT
                dbg_sums = None
                if os.environ.get("KERNEL_DEBUG") and tqb == NTQB - 1:
                    dbg_sums = persist2.tile([97, TQB], F32)
                    nc.scalar.copy(dbg_sums[:], sums[:])
                for h in range(HPC):
                    # reciprocal_approx_fast mis-reads PSUM partition offsets
                    # 32/64/96 on HW: stage the row to partition 0 first
                    sr = smp.tile([1, TQB], F32, tag="sr")
                    nc.scalar.copy(sr[:], sums[32 * h:32 * h + 1, :])
                    rc = smp.tile([1, TQB], F32, tag="rc")
                    nc.vector.reciprocal_approx_fast(rc[:], sr[:])
                    rbc = rbcp.tile([P, TQB], F32, tag="rbc")
                    nc.gpsimd.partition_broadcast(rbc[:], rc[:])
                    if dbg_sums is not None:
                        dbg_rbcs.append(rbc)
                    nc.vector.tensor_mul(
                        aT_v[:, h, tq0:tq0 + TQB], o_ps[h][:], rbc[:])
                if dbg_sums is not None:
                    dbg_extra.append(("dbg_sums", dbg_sums))
                if tqb > 0:
                    emit_oproj(tqb - 1)
            emit_oproj(NTQB - 1)

            if os.environ.get("KERNEL_DEBUG"):
                for nm, t in (("dbg_qT", qT), ("dbg_kT", kT),
                              ("dbg_vT", vT), ("dbg_aT", aT),
                              ("dbg_em", em_s), ("dbg_ones4", ones4)):
                    dd = nc.dram_tensor(
                        nm, list(t[:].shape), F16, kind="ExternalOutput").ap()
                    nc.sync.dma_start(dd, t[:])
                for hh, rb in enumerate(dbg_rbcs):
                    dd = nc.dram_tensor(
                        f"dbg_rbc{hh}", [P, TQB], F32,
                        kind="ExternalOutput").ap()
                    nc.sync.dma_start(dd, rb[:])
                for nm, t in dbg_extra:
                    dd = nc.dram_tensor(
                        nm, list(t[:].shape), F32, kind="ExternalOutput").ap()
                    nc.sync.dma_start(dd, t[:])

    nc.compile()
    return nc


def _host_prep(hidden_states, attention_mask, position_ids):
    hs2 = np.asarray(hidden_states, dtype=np.float32).reshape(B * S, D)
    hsT = np.ascontiguousarray(hs2.T).astype(ml_dtypes.float16 if False
                                             else np.float16)  # [D, B*S]

    # rope tables gathered by position_ids, feature-major
    inv_freq = 1.0 / (ROPE_THETA ** (np.arange(0, HD, 2, dtype=np.float32) / HD))
    pos = np.asarray(position_ids).astype(np.int64)
    maxpos = int(pos.max()) + 1
    t_ar = np.arange(maxpos, dtype=np.float32)
    freqs = np.outer(t_ar, inv_freq)
    emb = np.concatenate([freqs, freqs], axis=-1)           # [maxpos, 128]
    cos_tab = np.cos(emb).astype(np.float32)
    sin_tab = np.sin(emb).astype(np.float32)
    cosT = [np.ascontiguousarray(cos_tab[pos[b]].T).astype(np.float16)
            for b in range(B)]                               # [HD, S] each
    sinT = [np.ascontiguousarray(sin_tab[pos[b]].T).astype(np.float16)
            for b in range(B)]

    # shifted-exp mask, transposed per batch, tile classification + dedup
    m = np.asarray(attention_mask, dtype=np.float32)[:, 0]  # [B, tq, tk]
    rowmax = m.max(axis=-1, keepdims=True)
    em = np.exp(m - rowmax)                                 # [B, tq, tk] in [0,1]
    emT = np.ascontiguousarray(em.transpose(0, 2, 1))       # [B, tk, tq]
    emr = emT.reshape(B, NTK, P, NTQB, TQB)
    tmax = emr.max(axis=(2, 4))                             # [B, NTK, NTQB]
    tmin = emr.min(axis=(2, 4))
    cls_b = np.full((B, NTK, NTQB), CLS_MIXED, dtype=np.int8)
    cls_b[tmax == 0.0] = CLS_SKIP
    cls_b[(tmin == 1.0) & (tmax == 1.0)] = CLS_ZERO
    # merge across batches: process if any batch needs it; mixed if classes
    # differ or any is mixed (em data is per-batch anyway)
    cls = np.maximum(cls_b[0], cls_b[1])
    cls[cls_b[0] != cls_b[1]] = CLS_MIXED
    # guard: a fully-skipped tq column would leave PSUM unwritten
    for tqb in range(NTQB):
        if (cls[:, tqb] == CLS_SKIP).all():
            cls[0, tqb] = CLS_MIXED

    # per-batch unique mixed-tile patterns, in (tqb, tk) scan order
    uidseqs, uniqs = [], []
    for b in range(B):
        seen = {}
        seq = []
        tiles = []
        for tqb in range(NTQB):
            for tk in range(NTK):
                if cls[tk, tqb] != CLS_MIXED:
                    continue
                blk = np.ascontiguousarray(
                    emT[b, tk * P:(tk + 1) * P,
                        tqb * TQB:(tqb + 1) * TQB]).astype(np.float16)
                key = blk.tobytes()
                if key not in seen:
                    seen[key] = len(tiles)
                    tiles.append(blk)
                seq.append(seen[key])
        uidseqs.append(tuple(seq))
        uniqs.append(tiles)
    if uidseqs[0] != uidseqs[1]:
        # fall back to no dedup: sequential uids shared by construction
        nm = len(uidseqs[0])
        seqs = tuple(range(nm))
        uidseqs = [seqs, seqs]
        uniqs = [
            [np.ascontiguousarray(
                emT[b, tk * P:(tk + 1) * P, tqb * TQB:(tqb + 1) * TQB]
             ).astype(np.float16)
             for tqb in range(NTQB) for tk in range(NTK)
             if cls[tk, tqb] == CLS_MIXED]
            for b in range(B)
        ]
    n_uniq = max(len(uniqs[0]), len(uniqs[1]), 1)
    em_u = []
    for b in range(B):
        buf = np.zeros((P, n_uniq * TQB), dtype=np.float16)
        for u, blk in enumerate(uniqs[b]):
            buf[:, u * TQB:(u + 1) * TQB] = blk
        em_u.append(buf)
    return hsT, cosT, sinT, cls, uidseqs[0], n_uniq, em_u


def _sb_w(w):  # [D, JC] fp32 -> SBUF layout [128, KT*JC] fp16
    return np.ascontiguousarray(
        w.reshape(KT, P, JC).transpose(1, 0, 2).reshape(P, KT * JC)
    ).astype(np.float16)


def kernel(hidden_states, attention_mask, position_ids, Wq, Wk, Wv, Wo):
    hsT, cosT, sinT, cls, uidseq, n_uniq, em_u = _host_prep(
        hidden_states, attention_mask, position_ids)

    key = (cls.tobytes(), uidseq, n_uniq)
    if key not in _prog_cache:
        _prog_cache[key] = _build_program(cls, uidseq, n_uniq)
    nc = _prog_cache[key]

    Wq = np.asarray(Wq, dtype=np.float32)
    Wk = np.asarray(Wk, dtype=np.float32)
    Wv = np.asarray(Wv, dtype=np.float32)
    Wo = np.asarray(Wo, dtype=np.float32)

    in_maps = []
    for c in range(NCORES):
        b, g = c // CPG, c % CPG
        jsl = slice(g * JC, (g + 1) * JC)
        m = {
            "hsT": np.ascontiguousarray(hsT[:, b * S:(b + 1) * S]),
            "wq": _sb_w(Wq[:, jsl]),
            "wk": _sb_w(Wk[:, jsl]),
            "wv": _sb_w(Wv[:, jsl]),
            "wo": np.ascontiguousarray(
                Wo[jsl, :].reshape(HPC, P, D).transpose(1, 0, 2)
                .reshape(P, HPC * D)).astype(np.float16),
            "cosT": cosT[b],
            "sinT": sinT[b],
            "emU": em_u[b],
        }
        in_maps.append(m)

    if os.environ.get("KERNEL_SIM"):
        from concourse.bass_interp import CoreSim
        outs = []
        for c in range(int(os.environ.get("KERNEL_SIM_CORES", "1"))):
            sim = CoreSim(nc, require_finite=False, require_nnan=True)
            for k, v in in_maps[c].items():
                sim.tensor(k)[:] = v
            sim.simulate(check_with_hw=False)
            outs.append(np.array(sim.tensor("out")).astype(np.float32))
        kernel.last_sim_partials = outs
        total = np.zeros((B, S, D), dtype=np.float32)
        for c, o in enumerate(outs):
            total[c // CPG] += o
        return total

    from concourse.bass_utils import run_bass_kernel_spmd
    trace = bool(os.environ.get("KERNEL_TRACE"))
    res = run_bass_kernel_spmd(
        nc, in_maps, core_ids=list(range(NCORES)), trace=trace)
    if trace and res.exec_time_ns is not None:
        print(f"HW exec time: {res.exec_time_ns} ns")
        kernel.last_exec_time_ns = res.exec_time_ns
        kernel.last_trace = res.instructions_and_trace
    total = np.zeros((B, S, D), dtype=np.float32)
    for c, r in enumerate(res.results):
        total[c // CPG] += np.asarray(r["out"], dtype=np.float32)
    return total


# revision 22
# speedup vs baseline: 1.3478x; 1.0804x over previous
"""Trainium2 Bass kernel for LlamaAttention (B=2, S=2048, D=2048, H=16, HD=128).

Sharding: batch-split x head tensor-parallel. Cores 0-3 take batch 0, cores
4-7 batch 1; within a group each core owns 4 heads (512 feature columns of
Wq/Wk/Wv, 512 rows of Wo). Each core computes q/k/v projections + rope for
its heads, causal-masked softmax attention, AV, and a partial output
projection; the host sums the 4 partials per batch.

All matmul operands are fp16 (PSUM accumulation stays fp32; the softmax
denominator pipeline is fp32). Device layout is feature-major: hs^T [D, S]
so contraction dims land on SBUF partitions; attention runs on
scores^T = k-block^T @ q^T tiles. Softmax uses the host-shifted mask trick
(P = exp(scale*S) * em with em in [0,1]); row sums via a PE ones-matmul
into partition-sliced rows of one PSUM bank; normalization via
reciprocal_approx_fast + gpsimd broadcast folded into the PSUM eviction.
V never spills to DRAM: it is PE-transposed and kept resident in SBUF.
The unique causal-boundary mask tiles (4 patterns) are loaded once and
kept resident. Output projection for token block i is emitted while
attention for block i+1 runs, keeping the PE queue dense.
"""

import os
import sys
from contextlib import ExitStack

import numpy as np

for _p in ("/opt/trn_rl_repo",):
    if _p not in sys.path:
        sys.path.insert(0, _p)

import ml_dtypes  # noqa: E402

import concourse.bass as bass  # noqa: E402,F401
import concourse.tile as tile  # noqa: E402
from concourse import bacc, mybir  # noqa: E402
from concourse.masks import make_identity  # noqa: E402

B, S, D, H, HD = 2, 2048, 2048, 16, 128
NCORES = 8
CPG = 4                      # cores per batch group
HPC = H // CPG               # 4 heads per core
JC = HPC * HD                # 512 per-core feature columns
P = 128
TB = 512                     # token block for projections
NTB = S // TB                # 4 per core (one batch)
KT = D // P                  # 16 contraction tiles
TQB = 512                    # tq block in attention
NTQB = S // TQB              # 4
NTK = S // P                 # 16 tk tiles
NBW = 256                    # output-projection free-dim block
SCALE = 1.0 / float(np.sqrt(HD))
ROPE_THETA = 10000.0

F32 = mybir.dt.float32
F16 = mybir.dt.float16

# tile classes
CLS_SKIP, CLS_ZERO, CLS_MIXED = 0, 1, 2

_prog_cache: dict[tuple, object] = {}


def _build_program(cls: np.ndarray, uidseq: tuple, n_uniq: int):
    """cls: [NTK, NTQB] int8 tile classes (shared by both batches).
    uidseq: for each mixed tile in (tqb, tk) scan order, the index of its
    mask pattern inside the resident em tensor."""
    nc = bacc.Bacc(
        "TRN2",
        target_bir_lowering=False,
        debug=False,
        enable_asserts=True,
        num_devices=NCORES,
    )

    hsT_d = nc.dram_tensor("hsT", [D, S], F16, kind="ExternalInput").ap()
    wq_d = nc.dram_tensor("wq", [P, KT * JC], F16, kind="ExternalInput").ap()
    wk_d = nc.dram_tensor("wk", [P, KT * JC], F16, kind="ExternalInput").ap()
    wv_d = nc.dram_tensor("wv", [P, KT * JC], F16, kind="ExternalInput").ap()
    wo_d = nc.dram_tensor("wo", [P, HPC * D], F16, kind="ExternalInput").ap()
    cos_d = nc.dram_tensor("cosT", [HD, S], F16, kind="ExternalInput").ap()
    sin_d = nc.dram_tensor("sinT", [HD, S], F16, kind="ExternalInput").ap()
    em_d = nc.dram_tensor("emU", [P, max(n_uniq, 1) * TQB], F16,
                          kind="ExternalInput").ap()
    out_d = nc.dram_tensor("out", [S, D], F16, kind="ExternalOutput").ap()

    hsT_v = hsT_d.rearrange("(kt p) t -> p kt t", p=P)       # [128, 16, 2048]

    # mixed-tile uid lookup in (tqb, tk) scan order
    uid_of = {}
    ui = 0
    for tqb in range(NTQB):
        for tk in range(NTK):
            if cls[tk, tqb] == CLS_MIXED:
                uid_of[(tk, tqb)] = uidseq[ui]
                ui += 1
    assert ui == len(uidseq)

    FH = 2                       # heads per attention flight
    NFL = HPC // FH              # 2 flights

    with tile.TileContext(nc) as tc, ExitStack() as ctx:
        # ---------- long-lived tiles ----------
        persist = ctx.enter_context(tc.tile_pool(name="persist", bufs=1))
        qT = persist.tile([P, HPC * S], F16)      # [hd, (h, t)]
        kT = persist.tile([P, HPC * S], F16)
        vT = persist.tile([P, HPC * NTK * P], F16)  # [tok%128, (h, tk, hd)]
        ident = persist.tile([P, P], F16)
        make_identity(nc, ident[:])
        # ones2[:, hh, :]: column 32*hh all-ones. Lands flight-head hh's
        # exp-sum on partition 32*hh of a shared [33, TQB] PSUM region
        # (engines may only address partition offsets 0/32/64/96).
        ones2 = persist.tile([P, FH * 33], F16)
        nc.any.memset(ones2[:], 0.0)
        ones2_v = ones2[:].rearrange("p (h c) -> p h c", h=FH)
        for hh in range(FH):
            nc.any.memset(ones2_v[:, hh, 32 * hh:32 * hh + 1], 1.0)

        qT_v = qT[:].rearrange("p (h t) -> p h t", h=HPC)
        kT_v = kT[:].rearrange("p (h t) -> p h t", h=HPC)
        vT_v = vT[:].rearrange("p (h m j) -> p h m j", h=HPC, m=NTK)

        aT = persist.tile([P, HPC * S], F16)      # attn out^T [hd, (h, t)]
        aT_v = aT[:].rearrange("p (h t) -> p h t", h=HPC)
        wo_s = persist.tile([P, HPC * D], F16)
        nc.sync.dma_start(wo_s[:], wo_d)
        wo_sv = wo_s[:].rearrange("p (h n) -> p h n", h=HPC)
        em_s = persist.tile([P, max(n_uniq, 1) * TQB], F16)
        nc.sync.dma_start(em_s[:], em_d)

        # ---------- phase 1: q/k/v projections (+rope, v transpose) ----------
        with tc.tile_pool(name="wpool", bufs=1) as wpool, \
             tc.tile_pool(name="cspool", bufs=1) as cspool, \
             tc.tile_pool(name="hstp", bufs=2) as hstp, \
             tc.tile_pool(name="stg", bufs=2) as stg, \
             tc.tile_pool(name="vstg", bufs=HPC) as vstgp, \
             tc.tile_pool(name="ppsum", bufs=2, space="PSUM") as pps, \
             tc.tile_pool(name="vtpsum", bufs=2, space="PSUM") as vtp:
            wq_s = wpool.tile([P, KT * JC], F16)
            wk_s = wpool.tile([P, KT * JC], F16)
            wv_s = wpool.tile([P, KT * JC], F16)
            for w_s, w_dd in ((wv_s, wv_d), (wq_s, wq_d), (wk_s, wk_d)):
                nc.sync.dma_start(w_s[:], w_dd)
            cos_s = cspool.tile([HD, S], F16)
            sin_s = cspool.tile([HD, S], F16)
            nc.sync.dma_start(cos_s[:], cos_d)
            nc.sync.dma_start(sin_s[:], sin_d)

            w_views = {
                w: ws[:].rearrange("p (kt j) -> p kt j", j=JC)
                for w, ws in (("q", wq_s), ("k", wk_s), ("v", wv_s))
            }

            def proj_group(wname, j2, hst_v, dst_ps):
                for kt in range(KT):
                    nc.tensor.matmul(
                        dst_ps[:],
                        lhsT=w_views[wname][:, kt, j2 * P:(j2 + 1) * P],
                        rhs=hst_v[:, kt, :],
                        start=(kt == 0),
                        stop=(kt == KT - 1),
                    )

            def rope_evict(j2, dst, ps, tsl):
                # out[:64] = x1*cos - x2*sin ; out[64:] = x2*cos + x1*sin
                c1 = stg.tile([P, TB], F32, tag="ropeA")
                c2 = stg.tile([P, TB], F32, tag="ropeB")
                nc.vector.tensor_mul(c1[:], ps[:], cos_s[:, tsl])
                nc.vector.tensor_mul(
                    c2[0:64, :], ps[64:128, :], sin_s[0:64, tsl])
                nc.vector.tensor_mul(
                    c2[64:128, :], ps[0:64, :], sin_s[64:128, tsl])
                nc.vector.tensor_sub(dst[0:64, :], c1[0:64, :], c2[0:64, :])
                nc.vector.tensor_add(
                    dst[64:128, :], c1[64:128, :], c2[64:128, :])

            for tb in range(NTB):
                hst = hstp.tile([P, KT * TB], F16, tag="hst")
                for kt in range(KT):
                    nc.sync.dma_start(
                        hst[:, kt * TB:(kt + 1) * TB],
                        hsT_v[:, kt, tb * TB:(tb + 1) * TB],
                    )
                hst_v = hst[:].rearrange("p (kt t) -> p kt t", t=TB)
                tsl = slice(tb * TB, (tb + 1) * TB)
                # v first: its fp16 staging copies run while q projects, so
                # the PE transposes (emitted after q) never wait
                vss = []
                for j2 in range(HPC):
                    ps = pps.tile([P, TB], F32, tag="pp")
                    proj_group("v", j2, hst_v, ps)
                    vs = vstgp.tile([P, TB], F16, tag="vstage")
                    nc.scalar.copy(vs[:], ps[:])
                    vss.append(vs)
                for j2 in range(HPC):
                    ps = pps.tile([P, TB], F32, tag="pp")
                    proj_group("q", j2, hst_v, ps)
                    rope_evict(j2, qT_v[:, j2, tsl], ps, tsl)
                for j2 in range(HPC):
                    for k in range(TB // P):
                        vps = vtp.tile([P, P], F16, tag="vt")
                        nc.tensor.matmul(
                            vps[:],
                            lhsT=vss[j2][:, k * P:(k + 1) * P],
                            rhs=ident[:],
                            is_transpose=True,
                        )
                        nc.scalar.copy(
                            vT_v[:, j2, tb * (TB // P) + k, :], vps[:])
                for j2 in range(HPC):
                    ps = pps.tile([P, TB], F32, tag="pp")
                    proj_group("k", j2, hst_v, ps)
                    rope_evict(j2, kT_v[:, j2, tsl], ps, tsl)

        # ---------- phase 2: attention with interleaved output projection ----
        with tc.tile_pool(name="ptp", bufs=8) as ptp, \
             tc.tile_pool(name="ptmp", bufs=8) as ptmp, \
             tc.tile_pool(name="smp", bufs=4) as smp, \
             tc.tile_pool(name="rbcp", bufs=4) as rbcp, \
             tc.tile_pool(name="ostg", bufs=3) as ostgp, \
             tc.tile_pool(name="spsum", bufs=2, space="PSUM") as sps, \
             tc.tile_pool(name="opsum", bufs=2, space="PSUM") as ops, \
             tc.tile_pool(name="avpsum", bufs=1, space="PSUM") as avp, \
             tc.tile_pool(name="supsum", bufs=2, space="PSUM") as sups:
            o_ps = [avp.tile([P, TQB], F32, tag=f"av{hh}", name=f"o_ps{hh}")
                    for hh in range(FH)]
            dbg_rbcs: list = []
            dbg_extra: list = []

            pending: list = []   # (t, tb32, nb) output-projection groups
            ng = [0]             # running group counter for engine alternation

            def emit_oproj_groups(n):
                for _ in range(min(n, len(pending))):
                    t, tb32, nb = pending.pop(0)
                    pso = ops.tile([P, TQB], F32, tag="op")
                    for j2 in range(HPC):
                        nc.tensor.matmul(
                            pso[:],
                            lhsT=aT_v[:, j2, tb32 * P:(tb32 + 1) * P],
                            rhs=wo_sv[:, j2, nb * TQB:(nb + 1) * TQB],
                            start=(j2 == 0), stop=(j2 == HPC - 1),
                        )
                    og = ostgp.tile([P, TQB], F16, tag="og")
                    # alternate eviction engine to balance scalar/vector load
                    if ng[0] % 2 == 0:
                        nc.vector.tensor_copy(og[:], pso[:])
                    else:
                        nc.scalar.copy(og[:], pso[:])
                    ng[0] += 1
                    nc.sync.dma_start(
                        out_d[tb32 * P:(tb32 + 1) * P,
                              nb * TQB:(nb + 1) * TQB],
                        og[:],
                    )

            for tqb in range(NTQB):
                tq0 = tqb * TQB
                live = [tk for tk in range(NTK) if cls[tk, tqb] != CLS_SKIP]
                L = len(live)
                for fl in range(NFL):
                    heads = range(fl * FH, (fl + 1) * FH)
                    sums = sups.tile([33, TQB], F32, tag="sums")
                    pts = [[None] * FH for _ in range(2)]
                    for i, tk in enumerate(live):
                        for hh, h in enumerate(heads):
                            st = sps.tile([P, TQB], F32, tag="st")
                            nc.tensor.matmul(
                                st[:],
                                lhsT=kT_v[:, h, tk * P:(tk + 1) * P],
                                rhs=qT_v[:, h, tq0:tq0 + TQB],
                                start=True, stop=True,
                            )
                            pt = ptp.tile([P, TQB], F16, tag="pt")
                            nc.scalar.activation(
                                pt[:], st[:],
                                mybir.ActivationFunctionType.Exp,
                                scale=SCALE,
                            )
                            if cls[tk, tqb] == CLS_MIXED:
                                # out-of-place: PE never sees pre-mask pt
                                u = uid_of[(tk, tqb)]
                                ptm = ptmp.tile([P, TQB], F16, tag="ptm")
                                nc.vector.tensor_mul(
                                    ptm[:], pt[:],
                                    em_s[:, u * TQB:(u + 1) * TQB])
                                pt = ptm
                            pts[i % 2][hh] = pt
                            if i > 0:
                                pt_prev = pts[(i - 1) % 2][hh]
                                nc.tensor.matmul(
                                    o_ps[hh][:],
                                    lhsT=vT_v[:, h, live[i - 1], :],
                                    rhs=pt_prev[:],
                                    start=(i - 1 == 0), stop=False,
                                )
                                nc.tensor.matmul(
                                    sums[:],
                                    lhsT=ones2_v[:, hh, :],
                                    rhs=pt_prev[:],
                                    start=(i - 1 == 0 and hh == 0),
                                    stop=False,
                                )
                    for hh, h in enumerate(heads):
                        pt_last = pts[(L - 1) % 2][hh]
                        nc.tensor.matmul(
                            o_ps[hh][:],
                            lhsT=vT_v[:, h, live[L - 1], :],
                            rhs=pt_last[:],
                            start=(L == 1), stop=True,
                        )
                        nc.tensor.matmul(
                            sums[:],
                            lhsT=ones2_v[:, hh, :],
                            rhs=pt_last[:],
                            start=(L == 1 and hh == 0), stop=(hh == FH - 1),
                        )
                    dbg_sums = None
                    if os.environ.get("KERNEL_DEBUG") and tqb == NTQB - 1:
                        dbg_sums = persist.tile([33, TQB], F32)
                        nc.scalar.copy(dbg_sums[:], sums[:])
                        dbg_extra.append((f"dbg_sums{fl}", dbg_sums))
                    for hh, h in enumerate(heads):
                        # reciprocal_approx_fast mis-reads PSUM partition
                        # offsets != 0 on HW: stage the row to partition 0
                        sr = smp.tile([1, TQB], F32, tag="sr")
                        nc.scalar.copy(sr[:], sums[32 * hh:32 * hh + 1, :])
                        rc = smp.tile([1, TQB], F32, tag="rc")
                        nc.vector.reciprocal_approx_fast(rc[:], sr[:])
                        rbc = rbcp.tile([P, TQB], F32, tag="rbc")
                        nc.gpsimd.partition_broadcast(rbc[:], rc[:])
                        if dbg_sums is not None:
                            dbg_rbcs.append(rbc)
                        nc.vector.tensor_mul(
                            aT_v[:, h, tq0:tq0 + TQB], o_ps[hh][:], rbc[:])
                    emit_oproj_groups(8)
                pending += [(tqb, tb32, nb)
                            for tb32 in range(tqb * (TQB // P),
                                              (tqb + 1) * (TQB // P))
                            for nb in range(D // TQB)]
            emit_oproj_groups(len(pending))

            if os.environ.get("KERNEL_DEBUG"):
                for nm, t in (("dbg_qT", qT), ("dbg_kT", kT),
                              ("dbg_vT", vT), ("dbg_aT", aT),
                              ("dbg_em", em_s)):
                    dd = nc.dram_tensor(
                        nm, list(t[:].shape), F16, kind="ExternalOutput").ap()
                    nc.sync.dma_start(dd, t[:])
                for hh, rb in enumerate(dbg_rbcs):
                    dd = nc.dram_tensor(
                        f"dbg_rbc{hh}", [P, TQB], F32,
                        kind="ExternalOutput").ap()
                    nc.sync.dma_start(dd, rb[:])
                for nm, t in dbg_extra:
                    dd = nc.dram_tensor(
                        nm, list(t[:].shape), F32, kind="ExternalOutput").ap()
                    nc.sync.dma_start(dd, t[:])

    nc.compile()
    return nc


def _host_prep(hidden_states, attention_mask, position_ids):
    hs2 = np.asarray(hidden_states, dtype=np.float32).reshape(B * S, D)
    hsT = np.ascontiguousarray(hs2.T).astype(ml_dtypes.float16 if False
                                             else np.float16)  # [D, B*S]

    # rope tables gathered by position_ids, feature-major
    inv_freq = 1.0 / (ROPE_THETA ** (np.arange(0, HD, 2, dtype=np.float32) / HD))
    pos = np.asarray(position_ids).astype(np.int64)
    maxpos = int(pos.max()) + 1
    t_ar = np.arange(maxpos, dtype=np.float32)
    freqs = np.outer(t_ar, inv_freq)
    emb = np.concatenate([freqs, freqs], axis=-1)           # [maxpos, 128]
    cos_tab = np.cos(emb).astype(np.float32)
    sin_tab = np.sin(emb).astype(np.float32)
    cosT = [np.ascontiguousarray(cos_tab[pos[b]].T).astype(np.float16)
            for b in range(B)]                               # [HD, S] each
    sinT = [np.ascontiguousarray(sin_tab[pos[b]].T).astype(np.float16)
            for b in range(B)]

    # shifted-exp mask, transposed per batch, tile classification + dedup
    m = np.asarray(attention_mask, dtype=np.float32)[:, 0]  # [B, tq, tk]
    rowmax = m.max(axis=-1, keepdims=True)
    em = np.exp(m - rowmax)                                 # [B, tq, tk] in [0,1]
    emT = np.ascontiguousarray(em.transpose(0, 2, 1))       # [B, tk, tq]
    emr = emT.reshape(B, NTK, P, NTQB, TQB)
    tmax = emr.max(axis=(2, 4))                             # [B, NTK, NTQB]
    tmin = emr.min(axis=(2, 4))
    cls_b = np.full((B, NTK, NTQB), CLS_MIXED, dtype=np.int8)
    cls_b[tmax == 0.0] = CLS_SKIP
    cls_b[(tmin == 1.0) & (tmax == 1.0)] = CLS_ZERO
    # merge across batches: process if any batch needs it; mixed if classes
    # differ or any is mixed (em data is per-batch anyway)
    cls = np.maximum(cls_b[0], cls_b[1])
    cls[cls_b[0] != cls_b[1]] = CLS_MIXED
    # guard: a fully-skipped tq column would leave PSUM unwritten
    for tqb in range(NTQB):
        if (cls[:, tqb] == CLS_SKIP).all():
            cls[0, tqb] = CLS_MIXED

    # per-batch unique mixed-tile patterns, in (tqb, tk) scan order
    uidseqs, uniqs = [], []
    for b in range(B):
        seen = {}
        seq = []
        tiles = []
        for tqb in range(NTQB):
            for tk in range(NTK):
                if cls[tk, tqb] != CLS_MIXED:
                    continue
                blk = np.ascontiguousarray(
                    emT[b, tk * P:(tk + 1) * P,
                        tqb * TQB:(tqb + 1) * TQB]).astype(np.float16)
                key = blk.tobytes()
                if key not in seen:
                    seen[key] = len(tiles)
                    tiles.append(blk)
                seq.append(seen[key])
        uidseqs.append(tuple(seq))
        uniqs.append(tiles)
    if uidseqs[0] != uidseqs[1]:
        # fall back to no dedup: sequential uids shared by construction
        nm = len(uidseqs[0])
        seqs = tuple(range(nm))
        uidseqs = [seqs, seqs]
        uniqs = [
            [np.ascontiguousarray(
                emT[b, tk * P:(tk + 1) * P, tqb * TQB:(tqb + 1) * TQB]
             ).astype(np.float16)
             for tqb in range(NTQB) for tk in range(NTK)
             if cls[tk, tqb] == CLS_MIXED]
            for b in range(B)
        ]
    n_uniq = max(len(uniqs[0]), len(uniqs[1]), 1)
    em_u = []
    for b in range(B):
        buf = np.zeros((P, n_uniq * TQB), dtype=np.float16)
        for u, blk in enumerate(uniqs[b]):
            buf[:, u * TQB:(u + 1) * TQB] = blk
        em_u.append(buf)
    return hsT, cosT, sinT, cls, uidseqs[0], n_uniq, em_u


def _sb_w(w):  # [D, JC] fp32 -> SBUF layout [128, KT*JC] fp16
    return np.ascontiguousarray(
        w.reshape(KT, P, JC).transpose(1, 0, 2).reshape(P, KT * JC)
    ).astype(np.float16)


def kernel(hidden_states, attention_mask, position_ids, Wq, Wk, Wv, Wo):
    hsT, cosT, sinT, cls, uidseq, n_uniq, em_u = _host_prep(
        hidden_states, attention_mask, position_ids)

    key = (cls.tobytes(), uidseq, n_uniq)
    if key not in _prog_cache:
        _prog_cache[key] = _build_program(cls, uidseq, n_uniq)
    nc = _prog_cache[key]

    Wq = np.asarray(Wq, dtype=np.float32)
    Wk = np.asarray(Wk, dtype=np.float32)
    Wv = np.asarray(Wv, dtype=np.float32)
    Wo = np.asarray(Wo, dtype=np.float32)

    in_maps = []
    for c in range(NCORES):
        b, g = c // CPG, c % CPG
        jsl = slice(g * JC, (g + 1) * JC)
        m = {
            "hsT": np.ascontiguousarray(hsT[:, b * S:(b + 1) * S]),
            "wq": _sb_w(Wq[:, jsl]),
            "wk": _sb_w(Wk[:, jsl]),
            "wv": _sb_w(Wv[:, jsl]),
            "wo": np.ascontiguousarray(
                Wo[jsl, :].reshape(HPC, P, D).transpose(1, 0, 2)
                .reshape(P, HPC * D)).astype(np.float16),
            "cosT": cosT[b],
            "sinT": sinT[b],
            "emU": em_u[b],
        }
        in_maps.append(m)

    if os.environ.get("KERNEL_SIM"):
        from concourse.bass_interp import CoreSim
        outs = []
        for c in range(int(os.environ.get("KERNEL_SIM_CORES", "1"))):
            sim = CoreSim(nc, require_finite=False, require_nnan=True)
            for k, v in in_maps[c].items():
                sim.tensor(k)[:] = v
            sim.simulate(check_with_hw=False)
            outs.append(np.array(sim.tensor("out")).astype(np.float32))
        kernel.last_sim_partials = outs
        total = np.zeros((B, S, D), dtype=np.float32)
        for c, o in enumerate(outs):
            total[c // CPG] += o
        return total

    from concourse.bass_utils import run_bass_kernel_spmd
    trace = bool(os.environ.get("KERNEL_TRACE"))
    res = run_bass_kernel_spmd(
        nc, in_maps, core_ids=list(range(NCORES)), trace=trace)
    if trace and res.exec_time_ns is not None:
        print(f"HW exec time: {res.exec_time_ns} ns")
        kernel.last_exec_time_ns = res.exec_time_ns
        kernel.last_trace = res.instructions_and_trace
    total = np.zeros((B, S, D), dtype=np.float32)
    for c, r in enumerate(res.results):
        total[c // CPG] += np.asarray(r["out"], dtype=np.float32)
    return total


# revision 24
# speedup vs baseline: 1.4002x; 1.0389x over previous
"""Trainium2 Bass kernel for LlamaAttention (B=2, S=2048, D=2048, H=16, HD=128).

Sharding: batch-split x head tensor-parallel. Cores 0-3 take batch 0, cores
4-7 batch 1; within a group each core owns 4 heads (512 feature columns of
Wq/Wk/Wv, 512 rows of Wo). Each core computes q/k/v projections + rope for
its heads, causal-masked softmax attention, AV, and a partial output
projection; the host sums the 4 partials per batch.

All matmul operands are fp16 (PSUM accumulation stays fp32; the softmax
denominator pipeline is fp32). Device layout is feature-major: hs^T [D, S]
so contraction dims land on SBUF partitions; attention runs on
scores^T = k-block^T @ q^T tiles. Softmax uses the host-shifted mask trick
(P = exp(scale*S) * em with em in [0,1]); row sums via a PE ones-matmul
into partition-sliced rows of one PSUM bank; normalization via
reciprocal_approx_fast + gpsimd broadcast folded into the PSUM eviction.
V never spills to DRAM: it is PE-transposed and kept resident in SBUF.
The unique causal-boundary mask tiles (4 patterns) are loaded once and
kept resident. Output projection for token block i is emitted while
attention for block i+1 runs, keeping the PE queue dense.
"""

import os
import sys
from contextlib import ExitStack

import numpy as np

for _p in ("/opt/trn_rl_repo",):
    if _p not in sys.path:
        sys.path.insert(0, _p)

import ml_dtypes  # noqa: E402

import concourse.bass as bass  # noqa: E402,F401
import concourse.tile as tile  # noqa: E402
from concourse import bacc, mybir  # noqa: E402
from concourse.masks import make_identity  # noqa: E402

B, S, D, H, HD = 2, 2048, 2048, 16, 128
NCORES = 8
CPG = 4                      # cores per batch group
HPC = H // CPG               # 4 heads per core
JC = HPC * HD                # 512 per-core feature columns
P = 128
TB = 512                     # token block for projections
NTB = S // TB                # 4 per core (one batch)
KT = D // P                  # 16 contraction tiles
TQB = 512                    # tq block in attention
NTQB = S // TQB              # 4
NTK = S // P                 # 16 tk tiles
NBW = 256                    # output-projection free-dim block
SCALE = 1.0 / float(np.sqrt(HD))
ROPE_THETA = 10000.0

F32 = mybir.dt.float32
F16 = mybir.dt.float16

# tile classes
CLS_SKIP, CLS_ZERO, CLS_MIXED = 0, 1, 2

_prog_cache: dict[tuple, object] = {}


def _build_program(cls: np.ndarray, uidseq: tuple, n_uniq: int):
    """cls: [NTK, NTQB] int8 tile classes (shared by both batches).
    uidseq: for each mixed tile in (tqb, tk) scan order, the index of its
    mask pattern inside the resident em tensor."""
    nc = bacc.Bacc(
        "TRN2",
        target_bir_lowering=False,
        debug=False,
        enable_asserts=True,
        num_devices=NCORES,
    )

    hsT_d = nc.dram_tensor("hsT", [D, S], F16, kind="ExternalInput").ap()
    wq_d = nc.dram_tensor("wq", [P, KT * JC], F16, kind="ExternalInput").ap()
    wk_d = nc.dram_tensor("wk", [P, KT * JC], F16, kind="ExternalInput").ap()
    wv_d = nc.dram_tensor("wv", [P, KT * JC], F16, kind="ExternalInput").ap()
    wo_d = nc.dram_tensor("wo", [P, HPC * D], F16, kind="ExternalInput").ap()
    cos_d = nc.dram_tensor("cosT", [HD, S], F16, kind="ExternalInput").ap()
    sin_d = nc.dram_tensor("sinT", [HD, S], F16, kind="ExternalInput").ap()
    em_d = nc.dram_tensor("emU", [P, max(n_uniq, 1) * TQB], F16,
                          kind="ExternalInput").ap()
    out_d = nc.dram_tensor("out", [S, D], F16, kind="ExternalOutput").ap()

    hsT_v = hsT_d.rearrange("(kt p) t -> p kt t", p=P)       # [128, 16, 2048]

    # mixed-tile uid lookup in (tqb, tk) scan order
    uid_of = {}
    ui = 0
    for tqb in range(NTQB):
        for tk in range(NTK):
            if cls[tk, tqb] == CLS_MIXED:
                uid_of[(tk, tqb)] = uidseq[ui]
                ui += 1
    assert ui == len(uidseq)

    FH = 2                       # heads per attention flight
    NFL = HPC // FH              # 2 flights

    with tile.TileContext(nc) as tc, ExitStack() as ctx:
        # ---------- long-lived tiles ----------
        persist = ctx.enter_context(tc.tile_pool(name="persist", bufs=1))
        qT = persist.tile([P, HPC * S], F16)      # [hd, (h, t)]
        kT = persist.tile([P, HPC * S], F16)
        vT = persist.tile([P, HPC * NTK * P], F16)  # [tok%128, (h, tk, hd)]
        ident = persist.tile([P, P], F16)
        make_identity(nc, ident[:])
        # ones2[:, hh, :]: column 32*hh all-ones. Lands flight-head hh's
        # exp-sum on partition 32*hh of a shared [33, TQB] PSUM region
        # (engines may only address partition offsets 0/32/64/96).
        ones2 = persist.tile([P, FH * 33], F16)
        nc.any.memset(ones2[:], 0.0)
        ones2_v = ones2[:].rearrange("p (h c) -> p h c", h=FH)
        for hh in range(FH):
            nc.any.memset(ones2_v[:, hh, 32 * hh:32 * hh + 1], 1.0)

        qT_v = qT[:].rearrange("p (h t) -> p h t", h=HPC)
        kT_v = kT[:].rearrange("p (h t) -> p h t", h=HPC)
        vT_v = vT[:].rearrange("p (h m j) -> p h m j", h=HPC, m=NTK)

        aT = persist.tile([P, HPC * S], F16)      # attn out^T [hd, (h, t)]
        aT_v = aT[:].rearrange("p (h t) -> p h t", h=HPC)
        wo_s = persist.tile([P, HPC * D], F16)
        wo_sv = wo_s[:].rearrange("p (h n) -> p h n", h=HPC)
        em_s = persist.tile([P, max(n_uniq, 1) * TQB], F16)

        # ---------- phase 1: q/k/v projections (+rope, v transpose) ----------
        KQ = 4                  # kt tiles per quarter; deps are tile-granular
        NQ = KT // KQ
        with tc.tile_pool(name="wpool", bufs=1) as wpool, \
             tc.tile_pool(name="cspool", bufs=1) as cspool, \
             tc.tile_pool(name="hstp", bufs=2 * NQ) as hstp, \
             tc.tile_pool(name="stg", bufs=2) as stg, \
             tc.tile_pool(name="vstg", bufs=HPC) as vstgp, \
             tc.tile_pool(name="ppsum", bufs=3, space="PSUM") as pps, \
             tc.tile_pool(name="vtpsum", bufs=2, space="PSUM") as vtp:
            # weights and hs stream in kt-quarters so the PE can start after
            # ~1MB instead of waiting for whole-tile DMAs
            w_tiles = {w: [wpool.tile([P, KQ * JC], F16, name=f"w_{w}{qq}")
                           for qq in range(NQ)]
                       for w in ("v", "q", "k")}
            w_dram = {"v": wv_d, "q": wq_d, "k": wk_d}
            cos_s = cspool.tile([HD, S], F16)
            sin_s = cspool.tile([HD, S], F16)

            def load_hst_quarters(tb):
                qs = []
                for qq in range(NQ):
                    hq = hstp.tile([P, KQ * TB], F16, tag="hst")
                    for kl in range(KQ):
                        nc.sync.dma_start(
                            hq[:, kl * TB:(kl + 1) * TB],
                            hsT_v[:, qq * KQ + kl, tb * TB:(tb + 1) * TB],
                        )
                    qs.append(hq[:].rearrange("p (kt t) -> p kt t", t=TB))
                return qs

            hst_q = load_hst_quarters(0)
            for qq in range(NQ):
                nc.sync.dma_start(
                    w_tiles["v"][qq][:],
                    w_dram["v"][:, qq * KQ * JC:(qq + 1) * KQ * JC])
            nc.sync.dma_start(cos_s[:], cos_d)
            nc.sync.dma_start(sin_s[:], sin_d)
            for w in ("q", "k"):
                for qq in range(NQ):
                    nc.sync.dma_start(
                        w_tiles[w][qq][:],
                        w_dram[w][:, qq * KQ * JC:(qq + 1) * KQ * JC])
            nc.scalar.dma_start(wo_s[:], wo_d)
            nc.scalar.dma_start(em_s[:], em_d)

            w_views = {
                w: [t[:].rearrange("p (kt j) -> p kt j", j=JC)
                    for t in w_tiles[w]]
                for w in ("v", "q", "k")
            }

            def proj_group(wname, j2, hst_v, dst_ps):
                for kt in range(KT):
                    nc.tensor.matmul(
                        dst_ps[:],
                        lhsT=w_views[wname][kt // KQ][
                            :, kt % KQ, j2 * P:(j2 + 1) * P],
                        rhs=hst_v[kt // KQ][:, kt % KQ, :],
                        start=(kt == 0),
                        stop=(kt == KT - 1),
                    )

            def rope_evict(j2, dst, ps, tsl):
                # out[:64] = x1*cos - x2*sin ; out[64:] = x2*cos + x1*sin
                c1 = stg.tile([P, TB], F32, tag="ropeA")
                c2 = stg.tile([P, TB], F32, tag="ropeB")
                nc.vector.tensor_mul(c1[:], ps[:], cos_s[:, tsl])
                nc.vector.tensor_mul(
                    c2[0:64, :], ps[64:128, :], sin_s[0:64, tsl])
                nc.vector.tensor_mul(
                    c2[64:128, :], ps[0:64, :], sin_s[64:128, tsl])
                nc.vector.tensor_sub(dst[0:64, :], c1[0:64, :], c2[0:64, :])
                nc.vector.tensor_add(
                    dst[64:128, :], c1[64:128, :], c2[64:128, :])

            for tb in range(NTB):
                if tb > 0:
                    hst_q = load_hst_quarters(tb)
                tsl = slice(tb * TB, (tb + 1) * TB)
                # v first: its fp16 staging copies run while q projects, so
                # the PE transposes (emitted after q) never wait
                vss = []
                for j2 in range(HPC):
                    ps = pps.tile([P, TB], F32, tag="pp")
                    proj_group("v", j2, hst_q, ps)
                    vs = vstgp.tile([P, TB], F16, tag="vstage")
                    nc.scalar.copy(vs[:], ps[:])
                    vss.append(vs)
                for j2 in range(HPC):
                    ps = pps.tile([P, TB], F32, tag="pp")
                    proj_group("q", j2, hst_q, ps)
                    rope_evict(j2, qT_v[:, j2, tsl], ps, tsl)
                for j2 in range(HPC):
                    for k in range(TB // P):
                        vps = vtp.tile([P, P], F16, tag="vt")
                        nc.tensor.matmul(
                            vps[:],
                            lhsT=vss[j2][:, k * P:(k + 1) * P],
                            rhs=ident[:],
                            is_transpose=True,
                        )
                        nc.scalar.copy(
                            vT_v[:, j2, tb * (TB // P) + k, :], vps[:])
                for j2 in range(HPC):
                    ps = pps.tile([P, TB], F32, tag="pp")
                    proj_group("k", j2, hst_q, ps)
                    rope_evict(j2, kT_v[:, j2, tsl], ps, tsl)

        # ---------- phase 2: attention with interleaved output projection ----
        with tc.tile_pool(name="ptp", bufs=8) as ptp, \
             tc.tile_pool(name="ptmp", bufs=8) as ptmp, \
             tc.tile_pool(name="smp", bufs=4) as smp, \
             tc.tile_pool(name="rbcp", bufs=4) as rbcp, \
             tc.tile_pool(name="ostg", bufs=3) as ostgp, \
             tc.tile_pool(name="spsum", bufs=2, space="PSUM") as sps, \
             tc.tile_pool(name="opsum", bufs=2, space="PSUM") as ops, \
             tc.tile_pool(name="avpsum", bufs=1, space="PSUM") as avp, \
             tc.tile_pool(name="supsum", bufs=2, space="PSUM") as sups:
            o_ps = [avp.tile([P, TQB], F32, tag=f"av{hh}", name=f"o_ps{hh}")
                    for hh in range(FH)]
            dbg_rbcs: list = []
            dbg_extra: list = []

            pending: list = []   # (t, tb32, nb) output-projection groups
            ng = [0]             # running group counter for engine alternation

            def emit_oproj_groups(n):
                for _ in range(min(n, len(pending))):
                    t, tb32, nb = pending.pop(0)
                    pso = ops.tile([P, TQB], F32, tag="op")
                    for j2 in range(HPC):
                        nc.tensor.matmul(
                            pso[:],
                            lhsT=aT_v[:, j2, tb32 * P:(tb32 + 1) * P],
                            rhs=wo_sv[:, j2, nb * TQB:(nb + 1) * TQB],
                            start=(j2 == 0), stop=(j2 == HPC - 1),
                        )
                    og = ostgp.tile([P, TQB], F16, tag="og")
                    # alternate eviction engine to balance scalar/vector load
                    if ng[0] % 2 == 0:
                        nc.vector.tensor_copy(og[:], pso[:])
                    else:
                        nc.scalar.copy(og[:], pso[:])
                    ng[0] += 1
                    nc.sync.dma_start(
                        out_d[tb32 * P:(tb32 + 1) * P,
                              nb * TQB:(nb + 1) * TQB],
                        og[:],
                    )

            for tqb in range(NTQB):
                tq0 = tqb * TQB
                live = [tk for tk in range(NTK) if cls[tk, tqb] != CLS_SKIP]
                L = len(live)
                for fl in range(NFL):
                    heads = range(fl * FH, (fl + 1) * FH)
                    sums = sups.tile([33, TQB], F32, tag="sums")
                    pts = [[None] * FH for _ in range(2)]
                    for i, tk in enumerate(live):
                        for hh, h in enumerate(heads):
                            st = sps.tile([P, TQB], F32, tag="st")
                            nc.tensor.matmul(
                                st[:],
                                lhsT=kT_v[:, h, tk * P:(tk + 1) * P],
                                rhs=qT_v[:, h, tq0:tq0 + TQB],
                                start=True, stop=True,
                            )
                            pt = ptp.tile([P, TQB], F16, tag="pt")
                            nc.scalar.activation(
                                pt[:], st[:],
                                mybir.ActivationFunctionType.Exp,
                                scale=SCALE,
                            )
                            if cls[tk, tqb] == CLS_MIXED:
                                # out-of-place: PE never sees pre-mask pt
                                u = uid_of[(tk, tqb)]
                                ptm = ptmp.tile([P, TQB], F16, tag="ptm")
                                nc.vector.tensor_mul(
                                    ptm[:], pt[:],
                                    em_s[:, u * TQB:(u + 1) * TQB])
                                pt = ptm
                            pts[i % 2][hh] = pt
                            if i > 0:
                                pt_prev = pts[(i - 1) % 2][hh]
                                nc.tensor.matmul(
                                    o_ps[hh][:],
                                    lhsT=vT_v[:, h, live[i - 1], :],
                                    rhs=pt_prev[:],
                                    start=(i - 1 == 0), stop=False,
                                )
                                nc.tensor.matmul(
                                    sums[:],
                                    lhsT=ones2_v[:, hh, :],
                                    rhs=pt_prev[:],
                                    start=(i - 1 == 0 and hh == 0),
                                    stop=False,
                                )
                    for hh, h in enumerate(heads):
                        pt_last = pts[(L - 1) % 2][hh]
                        nc.tensor.matmul(
                            o_ps[hh][:],
                            lhsT=vT_v[:, h, live[L - 1], :],
                            rhs=pt_last[:],
                            start=(L == 1), stop=True,
                        )
                        nc.tensor.matmul(
                            sums[:],
                            lhsT=ones2_v[:, hh, :],
                            rhs=pt_last[:],
                            start=(L == 1 and hh == 0), stop=(hh == FH - 1),
                        )
                    dbg_sums = None
                    if os.environ.get("KERNEL_DEBUG") and tqb == NTQB - 1:
                        dbg_sums = persist.tile([33, TQB], F32)
                        nc.scalar.copy(dbg_sums[:], sums[:])
                        dbg_extra.append((f"dbg_sums{fl}", dbg_sums))
                    for hh, h in enumerate(heads):
                        # reciprocal_approx_fast mis-reads PSUM partition
                        # offsets != 0 on HW: stage the row to partition 0
                        sr = smp.tile([1, TQB], F32, tag="sr")
                        nc.scalar.copy(sr[:], sums[32 * hh:32 * hh + 1, :])
                        rc = smp.tile([1, TQB], F32, tag="rc")
                        nc.vector.reciprocal_approx_fast(rc[:], sr[:])
                        rbc = rbcp.tile([P, TQB], F32, tag="rbc")
                        nc.gpsimd.partition_broadcast(rbc[:], rc[:])
                        if dbg_sums is not None:
                            dbg_rbcs.append(rbc)
                        nc.vector.tensor_mul(
                            aT_v[:, h, tq0:tq0 + TQB], o_ps[hh][:], rbc[:])
                    emit_oproj_groups(8)
                pending += [(tqb, tb32, nb)
                            for tb32 in range(tqb * (TQB // P),
                                              (tqb + 1) * (TQB // P))
                            for nb in range(D // TQB)]
            emit_oproj_groups(len(pending))

            if os.environ.get("KERNEL_DEBUG"):
                for nm, t in (("dbg_qT", qT), ("dbg_kT", kT),
                              ("dbg_vT", vT), ("dbg_aT", aT),
                              ("dbg_em", em_s)):
                    dd = nc.dram_tensor(
                        nm, list(t[:].shape), F16, kind="ExternalOutput").ap()
                    nc.sync.dma_start(dd, t[:])
                for hh, rb in enumerate(dbg_rbcs):
                    dd = nc.dram_tensor(
                        f"dbg_rbc{hh}", [P, TQB], F32,
                        kind="ExternalOutput").ap()
                    nc.sync.dma_start(dd, rb[:])
                for nm, t in dbg_extra:
                    dd = nc.dram_tensor(
                        nm, list(t[:].shape), F32, kind="ExternalOutput").ap()
                    nc.sync.dma_start(dd, t[:])

    nc.compile()
    return nc


def _host_prep(hidden_states, attention_mask, position_ids):
    hs2 = np.asarray(hidden_states, dtype=np.float32).reshape(B * S, D)
    hsT = np.ascontiguousarray(hs2.T).astype(ml_dtypes.float16 if False
                                             else np.float16)  # [D, B*S]

    # rope tables gathered by position_ids, feature-major
    inv_freq = 1.0 / (ROPE_THETA ** (np.arange(0, HD, 2, dtype=np.float32) / HD))
    pos = np.asarray(position_ids).astype(np.int64)
    maxpos = int(pos.max()) + 1
    t_ar = np.arange(maxpos, dtype=np.float32)
    freqs = np.outer(t_ar, inv_freq)
    emb = np.concatenate([freqs, freqs], axis=-1)           # [maxpos, 128]
    cos_tab = np.cos(emb).astype(np.float32)
    sin_tab = np.sin(emb).astype(np.float32)
    cosT = [np.ascontiguousarray(cos_tab[pos[b]].T).astype(np.float16)
            for b in range(B)]                               # [HD, S] each
    sinT = [np.ascontiguousarray(sin_tab[pos[b]].T).astype(np.float16)
            for b in range(B)]

    # shifted-exp mask, transposed per batch, tile classification + dedup
    m = np.asarray(attention_mask, dtype=np.float32)[:, 0]  # [B, tq, tk]
    rowmax = m.max(axis=-1, keepdims=True)
    em = np.exp(m - rowmax)                                 # [B, tq, tk] in [0,1]
    emT = np.ascontiguousarray(em.transpose(0, 2, 1))       # [B, tk, tq]
    emr = emT.reshape(B, NTK, P, NTQB, TQB)
    tmax = emr.max(axis=(2, 4))                             # [B, NTK, NTQB]
    tmin = emr.min(axis=(2, 4))
    cls_b = np.full((B, NTK, NTQB), CLS_MIXED, dtype=np.int8)
    cls_b[tmax == 0.0] = CLS_SKIP
    cls_b[(tmin == 1.0) & (tmax == 1.0)] = CLS_ZERO
    # merge across batches: process if any batch needs it; mixed if classes
    # differ or any is mixed (em data is per-batch anyway)
    cls = np.maximum(cls_b[0], cls_b[1])
    cls[cls_b[0] != cls_b[1]] = CLS_MIXED
    # guard: a fully-skipped tq column would leave PSUM unwritten
    for tqb in range(NTQB):
        if (cls[:, tqb] == CLS_SKIP).all():
            cls[0, tqb] = CLS_MIXED

    # per-batch unique mixed-tile patterns, in (tqb, tk) scan order
    uidseqs, uniqs = [], []
    for b in range(B):
        seen = {}
        seq = []
        tiles = []
        for tqb in range(NTQB):
            for tk in range(NTK):
                if cls[tk, tqb] != CLS_MIXED:
                    continue
                blk = np.ascontiguousarray(
                    emT[b, tk * P:(tk + 1) * P,
                        tqb * TQB:(tqb + 1) * TQB]).astype(np.float16)
                key = blk.tobytes()
                if key not in seen:
                    seen[key] = len(tiles)
                    tiles.append(blk)
                seq.append(seen[key])
        uidseqs.append(tuple(seq))
        uniqs.append(tiles)
    if uidseqs[0] != uidseqs[1]:
        # fall back to no dedup: sequential uids shared by construction
        nm = len(uidseqs[0])
        seqs = tuple(range(nm))
        uidseqs = [seqs, seqs]
        uniqs = [
            [np.ascontiguousarray(
                emT[b, tk * P:(tk + 1) * P, tqb * TQB:(tqb + 1) * TQB]
             ).astype(np.float16)
             for tqb in range(NTQB) for tk in range(NTK)
             if cls[tk, tqb] == CLS_MIXED]
            for b in range(B)
        ]
    n_uniq = max(len(uniqs[0]), len(uniqs[1]), 1)
    em_u = []
    for b in range(B):
        buf = np.zeros((P, n_uniq * TQB), dtype=np.float16)
        for u, blk in enumerate(uniqs[b]):
            buf[:, u * TQB:(u + 1) * TQB] = blk
        em_u.append(buf)
    return hsT, cosT, sinT, cls, uidseqs[0], n_uniq, em_u


def _sb_w(w):  # [D, JC] fp32 -> SBUF layout [128, KT*JC] fp16
    return np.ascontiguousarray(
        w.reshape(KT, P, JC).transpose(1, 0, 2).reshape(P, KT * JC)
    ).astype(np.float16)


def kernel(hidden_states, attention_mask, position_ids, Wq, Wk, Wv, Wo):
    hsT, cosT, sinT, cls, uidseq, n_uniq, em_u = _host_prep(
        hidden_states, attention_mask, position_ids)

    key = (cls.tobytes(), uidseq, n_uniq)
    if key not in _prog_cache:
        _prog_cache[key] = _build_program(cls, uidseq, n_uniq)
    nc = _prog_cache[key]

    Wq = np.asarray(Wq, dtype=np.float32)
    Wk = np.asarray(Wk, dtype=np.float32)
    Wv = np.asarray(Wv, dtype=np.float32)
    Wo = np.asarray(Wo, dtype=np.float32)

    in_maps = []
    for c in range(NCORES):
        b, g = c // CPG, c % CPG
        jsl = slice(g * JC, (g + 1) * JC)
        m = {
            "hsT": np.ascontiguousarray(hsT[:, b * S:(b + 1) * S]),
            "wq": _sb_w(Wq[:, jsl]),
            "wk": _sb_w(Wk[:, jsl]),
            "wv": _sb_w(Wv[:, jsl]),
            "wo": np.ascontiguousarray(
                Wo[jsl, :].reshape(HPC, P, D).transpose(1, 0, 2)
                .reshape(P, HPC * D)).astype(np.float16),
            "cosT": cosT[b],
            "sinT": sinT[b],
            "emU": em_u[b],
        }
        in_maps.append(m)

    if os.environ.get("KERNEL_SIM"):
        from concourse.bass_interp import CoreSim
        outs = []
        for c in range(int(os.environ.get("KERNEL_SIM_CORES", "1"))):
            sim = CoreSim(nc, require_finite=False, require_nnan=True)
            for k, v in in_maps[c].items():
                sim.tensor(k)[:] = v
            sim.simulate(check_with_hw=False)
            outs.append(np.array(sim.tensor("out")).astype(np.float32))
        kernel.last_sim_partials = outs
        total = np.zeros((B, S, D), dtype=np.float32)
        for c, o in enumerate(outs):
            total[c // CPG] += o
        return total

    from concourse.bass_utils import run_bass_kernel_spmd
    trace = bool(os.environ.get("KERNEL_TRACE"))
    res = run_bass_kernel_spmd(
        nc, in_maps, core_ids=list(range(NCORES)), trace=trace)
    if trace and res.exec_time_ns is not None:
        print(f"HW exec time: {res.exec_time_ns} ns")
        kernel.last_exec_time_ns = res.exec_time_ns
        kernel.last_trace = res.instructions_and_trace
    total = np.zeros((B, S, D), dtype=np.float32)
    for c, r in enumerate(res.results):
        total[c // CPG] += np.asarray(r["out"], dtype=np.float32)
    return total


# revision 27
# speedup vs baseline: 1.4115x; 1.0080x over previous
"""Trainium2 Bass kernel for LlamaAttention (B=2, S=2048, D=2048, H=16, HD=128).

Sharding: batch-split x head tensor-parallel. Cores 0-3 take batch 0, cores
4-7 batch 1; within a group each core owns 4 heads (512 feature columns of
Wq/Wk/Wv, 512 rows of Wo). Each core computes q/k/v projections + rope for
its heads, causal-masked softmax attention, AV, and a partial output
projection; the host sums the 4 partials per batch.

All matmul operands are fp16 (PSUM accumulation stays fp32; the softmax
denominator pipeline is fp32). Device layout is feature-major: hs^T [D, S]
so contraction dims land on SBUF partitions; attention runs on
scores^T = k-block^T @ q^T tiles. Softmax uses the host-shifted mask trick
(P = exp(scale*S) * em with em in [0,1]); row sums via a PE ones-matmul
into partition-sliced rows of one PSUM bank; normalization via
reciprocal_approx_fast + gpsimd broadcast folded into the PSUM eviction.
V never spills to DRAM: it is PE-transposed and kept resident in SBUF.
The unique causal-boundary mask tiles (4 patterns) are loaded once and
kept resident. Output projection for token block i is emitted while
attention for block i+1 runs, keeping the PE queue dense.
"""

import os
import sys
from contextlib import ExitStack

import numpy as np

for _p in ("/opt/trn_rl_repo",):
    if _p not in sys.path:
        sys.path.insert(0, _p)

import ml_dtypes  # noqa: E402

import concourse.bass as bass  # noqa: E402,F401
import concourse.tile as tile  # noqa: E402
from concourse import bacc, mybir  # noqa: E402
from concourse.masks import make_identity  # noqa: E402

B, S, D, H, HD = 2, 2048, 2048, 16, 128
NCORES = 8
CPG = 4                      # cores per batch group
HPC = H // CPG               # 4 heads per core
JC = HPC * HD                # 512 per-core feature columns
P = 128
TB = 512                     # token block for projections
NTB = S // TB                # 4 per core (one batch)
KT = D // P                  # 16 contraction tiles
TQB = 512                    # tq block in attention
NTQB = S // TQB              # 4
NTK = S // P                 # 16 tk tiles
NBW = 256                    # output-projection free-dim block
SCALE = 1.0 / float(np.sqrt(HD))
ROPE_THETA = 10000.0

F32 = mybir.dt.float32
F16 = mybir.dt.float16

# tile classes
CLS_SKIP, CLS_ZERO, CLS_MIXED = 0, 1, 2

_prog_cache: dict[tuple, object] = {}


def _build_program(cls: np.ndarray, uidseq: tuple, n_uniq: int):
    """cls: [NTK, NTQB] int8 tile classes (shared by both batches).
    uidseq: for each mixed tile in (tqb, tk) scan order, the index of its
    mask pattern inside the resident em tensor."""
    nc = bacc.Bacc(
        "TRN2",
        target_bir_lowering=False,
        debug=False,
        enable_asserts=True,
        num_devices=NCORES,
    )

    hsT_d = nc.dram_tensor("hsT", [D, S], F16, kind="ExternalInput").ap()
    wq_d = nc.dram_tensor("wq", [P, KT * JC], F16, kind="ExternalInput").ap()
    wk_d = nc.dram_tensor("wk", [P, KT * JC], F16, kind="ExternalInput").ap()
    wv_d = nc.dram_tensor("wv", [P, KT * JC], F16, kind="ExternalInput").ap()
    wo_d = nc.dram_tensor("wo", [P, HPC * D], F16, kind="ExternalInput").ap()
    cos_d = nc.dram_tensor("cosT", [HD, S], F16, kind="ExternalInput").ap()
    sin_d = nc.dram_tensor("sinT", [HD, S], F16, kind="ExternalInput").ap()
    em_d = nc.dram_tensor("emU", [P, max(n_uniq, 1) * TQB], F16,
                          kind="ExternalInput").ap()
    out_d = nc.dram_tensor("out", [S, D], F16, kind="ExternalOutput").ap()

    hsT_v = hsT_d.rearrange("(kt p) t -> p kt t", p=P)       # [128, 16, 2048]

    # mixed-tile (uid, first-live-col) lookup in (tqb, tk) scan order
    uid_of = {}
    lo_of = {}
    ui = 0
    for tqb in range(NTQB):
        for tk in range(NTK):
            if cls[tk, tqb] == CLS_MIXED:
                uid_of[(tk, tqb)], lo_of[(tk, tqb)] = uidseq[ui]
                ui += 1
    assert ui == len(uidseq)

    FH = 2                       # heads per attention flight
    NFL = HPC // FH              # 2 flights

    with tile.TileContext(nc) as tc, ExitStack() as ctx:
        # ---------- long-lived tiles ----------
        persist = ctx.enter_context(tc.tile_pool(name="persist", bufs=1))
        qT = persist.tile([P, HPC * S], F16)      # [hd, (h, t)]
        kT = persist.tile([P, HPC * S], F16)
        vT = persist.tile([P, HPC * NTK * P], F16)  # [tok%128, (h, tk, hd)]
        ident = persist.tile([P, P], F16)
        make_identity(nc, ident[:])
        # ones2[:, hh, :]: column 32*hh all-ones. Lands flight-head hh's
        # exp-sum on partition 32*hh of a shared [33, TQB] PSUM region
        # (engines may only address partition offsets 0/32/64/96).
        ones2 = persist.tile([P, FH * 33], F16)
        nc.any.memset(ones2[:], 0.0)
        ones2_v = ones2[:].rearrange("p (h c) -> p h c", h=FH)
        for hh in range(FH):
            nc.any.memset(ones2_v[:, hh, 32 * hh:32 * hh + 1], 1.0)

        qT_v = qT[:].rearrange("p (h t) -> p h t", h=HPC)
        kT_v = kT[:].rearrange("p (h t) -> p h t", h=HPC)
        vT_v = vT[:].rearrange("p (h m j) -> p h m j", h=HPC, m=NTK)

        aT = persist.tile([P, HPC * S], F16)      # attn out^T [hd, (h, t)]
        aT_v = aT[:].rearrange("p (h t) -> p h t", h=HPC)
        wo_s = persist.tile([P, HPC * D], F16)
        wo_sv = wo_s[:].rearrange("p (h n) -> p h n", h=HPC)
        em_s = persist.tile([P, max(n_uniq, 1) * TQB], F16)

        # ---------- phase 1: q/k/v projections (+rope, v transpose) ----------
        KQ = 4                  # kt tiles per quarter; deps are tile-granular
        NQ = KT // KQ
        with tc.tile_pool(name="wpool", bufs=1) as wpool, \
             tc.tile_pool(name="cspool", bufs=1) as cspool, \
             tc.tile_pool(name="hstp", bufs=2 * NQ) as hstp, \
             tc.tile_pool(name="stg", bufs=2) as stg, \
             tc.tile_pool(name="vstg", bufs=HPC) as vstgp, \
             tc.tile_pool(name="ppsum", bufs=3, space="PSUM") as pps, \
             tc.tile_pool(name="vtpsum", bufs=2, space="PSUM") as vtp:
            # weights and hs stream in kt-quarters so the PE can start after
            # ~1MB instead of waiting for whole-tile DMAs
            w_tiles = {w: [wpool.tile([P, KQ * JC], F16, name=f"w_{w}{qq}")
                           for qq in range(NQ)]
                       for w in ("v", "q", "k")}
            w_dram = {"v": wv_d, "q": wq_d, "k": wk_d}
            cos_s = cspool.tile([HD, S], F16)
            sin_s = cspool.tile([HD, S], F16)

            def load_hst_quarter(tb, qq):
                hq = hstp.tile([P, KQ * TB], F16, tag="hst")
                nc.sync.dma_start(
                    hq[:].rearrange("p (kt t) -> p kt t", t=TB),
                    hsT_v[:, qq * KQ:(qq + 1) * KQ,
                          tb * TB:(tb + 1) * TB],
                )
                return hq[:].rearrange("p (kt t) -> p kt t", t=TB)

            def load_w_quarter(w, qq):
                nc.sync.dma_start(
                    w_tiles[w][qq][:],
                    w_dram[w][:, qq * KQ * JC:(qq + 1) * KQ * JC])

            # interleaved so the first v-projection group is gated on ~1MB
            hst_q = []
            for qq in range(NQ):
                hst_q.append(load_hst_quarter(0, qq))
                load_w_quarter("v", qq)
            nc.sync.dma_start(cos_s[:], cos_d)
            nc.sync.dma_start(sin_s[:], sin_d)
            for w in ("q", "k"):
                for qq in range(NQ):
                    load_w_quarter(w, qq)
            nc.scalar.dma_start(wo_s[:], wo_d)
            nc.scalar.dma_start(em_s[:], em_d)

            w_views = {
                w: [t[:].rearrange("p (kt j) -> p kt j", j=JC)
                    for t in w_tiles[w]]
                for w in ("v", "q", "k")
            }

            def proj_group(wname, j2, hst_v, dst_ps):
                for kt in range(KT):
                    nc.tensor.matmul(
                        dst_ps[:],
                        lhsT=w_views[wname][kt // KQ][
                            :, kt % KQ, j2 * P:(j2 + 1) * P],
                        rhs=hst_v[kt // KQ][:, kt % KQ, :],
                        start=(kt == 0),
                        stop=(kt == KT - 1),
                    )

            def rope_evict(j2, dst, ps, tsl):
                # out[:64] = x1*cos - x2*sin ; out[64:] = x2*cos + x1*sin
                c1 = stg.tile([P, TB], F32, tag="ropeA")
                c2 = stg.tile([P, TB], F32, tag="ropeB")
                nc.vector.tensor_mul(c1[:], ps[:], cos_s[:, tsl])
                nc.vector.tensor_mul(
                    c2[0:64, :], ps[64:128, :], sin_s[0:64, tsl])
                nc.vector.tensor_mul(
                    c2[64:128, :], ps[0:64, :], sin_s[64:128, tsl])
                nc.vector.tensor_sub(dst[0:64, :], c1[0:64, :], c2[0:64, :])
                nc.vector.tensor_add(
                    dst[64:128, :], c1[64:128, :], c2[64:128, :])

            for tb in range(NTB):
                if tb > 0:
                    hst_q = [load_hst_quarter(tb, qq) for qq in range(NQ)]
                tsl = slice(tb * TB, (tb + 1) * TB)
                # v first: its fp16 staging copies run while q projects, so
                # the PE transposes (emitted after q) never wait
                vss = []
                for j2 in range(HPC):
                    ps = pps.tile([P, TB], F32, tag="pp")
                    proj_group("v", j2, hst_q, ps)
                    vs = vstgp.tile([P, TB], F16, tag="vstage")
                    nc.scalar.copy(vs[:], ps[:])
                    vss.append(vs)
                for j2 in range(HPC):
                    ps = pps.tile([P, TB], F32, tag="pp")
                    proj_group("q", j2, hst_q, ps)
                    rope_evict(j2, qT_v[:, j2, tsl], ps, tsl)
                for j2 in range(HPC):
                    for k in range(TB // P):
                        vps = vtp.tile([P, P], F16, tag="vt")
                        nc.tensor.matmul(
                            vps[:],
                            lhsT=vss[j2][:, k * P:(k + 1) * P],
                            rhs=ident[:],
                            is_transpose=True,
                        )
                        nc.scalar.copy(
                            vT_v[:, j2, tb * (TB // P) + k, :], vps[:])
                for j2 in range(HPC):
                    ps = pps.tile([P, TB], F32, tag="pp")
                    proj_group("k", j2, hst_q, ps)
                    rope_evict(j2, kT_v[:, j2, tsl], ps, tsl)

        # ---------- phase 2: attention with interleaved output projection ----
        with tc.tile_pool(name="ptp", bufs=8) as ptp, \
             tc.tile_pool(name="ptmp", bufs=8) as ptmp, \
             tc.tile_pool(name="smp", bufs=4) as smp, \
             tc.tile_pool(name="rbcp", bufs=4) as rbcp, \
             tc.tile_pool(name="ostg", bufs=3) as ostgp, \
             tc.tile_pool(name="spsum", bufs=2, space="PSUM") as sps, \
             tc.tile_pool(name="opsum", bufs=2, space="PSUM") as ops, \
             tc.tile_pool(name="avpsum", bufs=1, space="PSUM") as avp, \
             tc.tile_pool(name="supsum", bufs=2, space="PSUM") as sups:
            o_ps = [avp.tile([P, TQB], F32, tag=f"av{hh}", name=f"o_ps{hh}")
                    for hh in range(FH)]
            dbg_rbcs: list = []
            dbg_extra: list = []

            pending: list = []   # (t, tb32, nb) output-projection groups
            ng = [0]             # running group counter for engine alternation

            def emit_oproj_groups(n):
                for _ in range(min(n, len(pending))):
                    t, tb32, nb = pending.pop(0)
                    pso = ops.tile([P, TQB], F32, tag="op")
                    for j2 in range(HPC):
                        nc.tensor.matmul(
                            pso[:],
                            lhsT=aT_v[:, j2, tb32 * P:(tb32 + 1) * P],
                            rhs=wo_sv[:, j2, nb * TQB:(nb + 1) * TQB],
                            start=(j2 == 0), stop=(j2 == HPC - 1),
                        )
                    og = ostgp.tile([P, TQB], F16, tag="og")
                    # alternate eviction engine to balance scalar/vector load
                    if ng[0] % 2 == 0:
                        nc.vector.tensor_copy(og[:], pso[:])
                    else:
                        nc.scalar.copy(og[:], pso[:])
                    ng[0] += 1
                    nc.sync.dma_start(
                        out_d[tb32 * P:(tb32 + 1) * P,
                              nb * TQB:(nb + 1) * TQB],
                        og[:],
                    )

            for tqb in range(NTQB):
                tq0 = tqb * TQB
                live = [tk for tk in range(NTK) if cls[tk, tqb] != CLS_SKIP]
                L = len(live)
                los = [lo_of.get((tk, tqb), 0) for tk in live]

                def av_sums(i, hh, h, pt, sums):
                    # one accumulation window per PSUM bank: tile 0 zeroes
                    # the full range (los[0] == 0 guaranteed), the last tile
                    # closes it; middle tiles write their live subrange only
                    first = (i == 0)
                    last = (i == L - 1)
                    lo = 0 if first else los[i]
                    nc.tensor.matmul(
                        o_ps[hh][:, lo:],
                        lhsT=vT_v[:, h, live[i], :],
                        rhs=pt[:, lo:],
                        start=first, stop=last,
                    )
                    nc.tensor.matmul(
                        sums[:, lo:],
                        lhsT=ones2_v[:, hh, :],
                        rhs=pt[:, lo:],
                        start=(first and hh == 0),
                        stop=(last and hh == FH - 1),
                    )

                for fl in range(NFL):
                    heads = range(fl * FH, (fl + 1) * FH)
                    sums = sups.tile([33, TQB], F32, tag="sums")
                    pts = [[None] * FH for _ in range(2)]
                    for i, tk in enumerate(live):
                        lo = los[i]
                        for hh, h in enumerate(heads):
                            st = sps.tile([P, TQB], F32, tag="st")
                            nc.tensor.matmul(
                                st[:, lo:],
                                lhsT=kT_v[:, h, tk * P:(tk + 1) * P],
                                rhs=qT_v[:, h, tq0 + lo:tq0 + TQB],
                                start=True, stop=True,
                            )
                            pt = ptp.tile([P, TQB], F16, tag="pt")
                            nc.scalar.activation(
                                pt[:, lo:], st[:, lo:],
                                mybir.ActivationFunctionType.Exp,
                                scale=SCALE,
                            )
                            if cls[tk, tqb] == CLS_MIXED:
                                # out-of-place: PE never sees pre-mask pt
                                u = uid_of[(tk, tqb)]
                                ptm = ptmp.tile([P, TQB], F16, tag="ptm")
                                nc.vector.tensor_mul(
                                    ptm[:, lo:], pt[:, lo:],
                                    em_s[:, u * TQB + lo:(u + 1) * TQB])
                                pt = ptm
                            pts[i % 2][hh] = pt
                            if i > 0:
                                av_sums(i - 1, hh, h,
                                        pts[(i - 1) % 2][hh], sums)
                    for hh, h in enumerate(heads):
                        av_sums(L - 1, hh, h, pts[(L - 1) % 2][hh], sums)
                    dbg_sums = None
                    if os.environ.get("KERNEL_DEBUG") and tqb == NTQB - 1:
                        dbg_sums = persist.tile([33, TQB], F32)
                        nc.scalar.copy(dbg_sums[:], sums[:])
                        dbg_extra.append((f"dbg_sums{fl}", dbg_sums))
                    for hh, h in enumerate(heads):
                        # reciprocal_approx_fast mis-reads PSUM partition
                        # offsets != 0 on HW: stage the row to partition 0
                        sr = smp.tile([1, TQB], F32, tag="sr")
                        nc.scalar.copy(sr[:], sums[32 * hh:32 * hh + 1, :])
                        rc = smp.tile([1, TQB], F32, tag="rc")
                        nc.vector.reciprocal_approx_fast(rc[:], sr[:])
                        rbc = rbcp.tile([P, TQB], F32, tag="rbc")
                        nc.gpsimd.partition_broadcast(rbc[:], rc[:])
                        if dbg_sums is not None:
                            dbg_rbcs.append(rbc)
                        nc.vector.tensor_mul(
                            aT_v[:, h, tq0:tq0 + TQB], o_ps[hh][:], rbc[:])
                    emit_oproj_groups(8)
                pending += [(tqb, tb32, nb)
                            for tb32 in range(tqb * (TQB // P),
                                              (tqb + 1) * (TQB // P))
                            for nb in range(D // TQB)]
            emit_oproj_groups(len(pending))

            if os.environ.get("KERNEL_DEBUG"):
                for nm, t in (("dbg_qT", qT), ("dbg_kT", kT),
                              ("dbg_vT", vT), ("dbg_aT", aT),
                              ("dbg_em", em_s)):
                    dd = nc.dram_tensor(
                        nm, list(t[:].shape), F16, kind="ExternalOutput").ap()
                    nc.sync.dma_start(dd, t[:])
                for hh, rb in enumerate(dbg_rbcs):
                    dd = nc.dram_tensor(
                        f"dbg_rbc{hh}", [P, TQB], F32,
                        kind="ExternalOutput").ap()
                    nc.sync.dma_start(dd, rb[:])
                for nm, t in dbg_extra:
                    dd = nc.dram_tensor(
                        nm, list(t[:].shape), F32, kind="ExternalOutput").ap()
                    nc.sync.dma_start(dd, t[:])

    nc.compile()
    return nc


def _host_prep(hidden_states, attention_mask, position_ids):
    hs2 = np.asarray(hidden_states, dtype=np.float32).reshape(B * S, D)
    hsT = np.ascontiguousarray(hs2.T).astype(ml_dtypes.float16 if False
                                             else np.float16)  # [D, B*S]

    # rope tables gathered by position_ids, feature-major
    inv_freq = 1.0 / (ROPE_THETA ** (np.arange(0, HD, 2, dtype=np.float32) / HD))
    pos = np.asarray(position_ids).astype(np.int64)
    maxpos = int(pos.max()) + 1
    t_ar = np.arange(maxpos, dtype=np.float32)
    freqs = np.outer(t_ar, inv_freq)
    emb = np.concatenate([freqs, freqs], axis=-1)           # [maxpos, 128]
    cos_tab = np.cos(emb).astype(np.float32)
    sin_tab = np.sin(emb).astype(np.float32)
    cosT = [np.ascontiguousarray(cos_tab[pos[b]].T).astype(np.float16)
            for b in range(B)]                               # [HD, S] each
    sinT = [np.ascontiguousarray(sin_tab[pos[b]].T).astype(np.float16)
            for b in range(B)]

    # shifted-exp mask, transposed per batch, tile classification + dedup
    m = np.asarray(attention_mask, dtype=np.float32)[:, 0]  # [B, tq, tk]
    rowmax = m.max(axis=-1, keepdims=True)
    em = np.exp(m - rowmax)                                 # [B, tq, tk] in [0,1]
    emT = np.ascontiguousarray(em.transpose(0, 2, 1))       # [B, tk, tq]
    emr = emT.reshape(B, NTK, P, NTQB, TQB)
    tmax = emr.max(axis=(2, 4))                             # [B, NTK, NTQB]
    tmin = emr.min(axis=(2, 4))
    cls_b = np.full((B, NTK, NTQB), CLS_MIXED, dtype=np.int8)
    cls_b[tmax == 0.0] = CLS_SKIP
    cls_b[(tmin == 1.0) & (tmax == 1.0)] = CLS_ZERO
    # merge across batches: process if any batch needs it; mixed if classes
    # differ or any is mixed (em data is per-batch anyway)
    cls = np.maximum(cls_b[0], cls_b[1])
    cls[cls_b[0] != cls_b[1]] = CLS_MIXED
    # guard: a fully-skipped tq column would leave PSUM unwritten
    for tqb in range(NTQB):
        if (cls[:, tqb] == CLS_SKIP).all():
            cls[0, tqb] = CLS_MIXED

    # per-batch unique mixed-tile patterns + first-live-column, scan order
    uidseqs, uniqs, loseqs = [], [], []
    for b in range(B):
        seen = {}
        seq = []
        los = []
        tiles = []
        for tqb in range(NTQB):
            for tk in range(NTK):
                if cls[tk, tqb] != CLS_MIXED:
                    continue
                blk = np.ascontiguousarray(
                    emT[b, tk * P:(tk + 1) * P,
                        tqb * TQB:(tqb + 1) * TQB]).astype(np.float16)
                key = blk.tobytes()
                if key not in seen:
                    seen[key] = len(tiles)
                    tiles.append(blk)
                seq.append(seen[key])
                nzc = blk.astype(bool).any(axis=0)
                los.append(int(np.argmax(nzc)) if nzc.any() else 0)
        uidseqs.append(tuple(seq))
        loseqs.append(los)
        uniqs.append(tiles)
    if uidseqs[0] != uidseqs[1]:
        # fall back to no dedup: sequential uids shared by construction
        nm = len(uidseqs[0])
        seqs = tuple(range(nm))
        uidseqs = [seqs, seqs]
        uniqs = [
            [np.ascontiguousarray(
                emT[b, tk * P:(tk + 1) * P, tqb * TQB:(tqb + 1) * TQB]
             ).astype(np.float16)
             for tqb in range(NTQB) for tk in range(NTK)
             if cls[tk, tqb] == CLS_MIXED]
            for b in range(B)
        ]
    # live-column offsets must agree across batches, else no restriction
    los = [a if a == bb else 0 for a, bb in zip(loseqs[0], loseqs[1])]
    # per tq block: offsets must start at 0 (zero-class tiles are implicit 0)
    # and be non-decreasing in tk scan order, else disable for that block
    ui = 0
    for tqb in range(NTQB):
        idxs = []
        full = []
        for tk in range(NTK):
            if cls[tk, tqb] == CLS_MIXED:
                full.append(los[ui])
                idxs.append(ui)
                ui += 1
            elif cls[tk, tqb] == CLS_ZERO:
                full.append(0)
        ok = all(full[i] <= full[i + 1] for i in range(len(full) - 1))
        if full and full[0] != 0:
            ok = False
        if not ok:
            for j in idxs:
                los[j] = 0
    uidseq = tuple(zip(uidseqs[0], los))
    n_uniq = max(len(uniqs[0]), len(uniqs[1]), 1)
    em_u = []
    for b in range(B):
        buf = np.zeros((P, n_uniq * TQB), dtype=np.float16)
        for u, blk in enumerate(uniqs[b]):
            buf[:, u * TQB:(u + 1) * TQB] = blk
        em_u.append(buf)
    return hsT, cosT, sinT, cls, uidseq, n_uniq, em_u


def _sb_w(w):  # [D, JC] fp32 -> SBUF layout [128, KT*JC] fp16
    return np.ascontiguousarray(
        w.reshape(KT, P, JC).transpose(1, 0, 2).reshape(P, KT * JC)
    ).astype(np.float16)


def kernel(hidden_states, attention_mask, position_ids, Wq, Wk, Wv, Wo):
    hsT, cosT, sinT, cls, uidseq, n_uniq, em_u = _host_prep(
        hidden_states, attention_mask, position_ids)

    key = (cls.tobytes(), uidseq, n_uniq)
    if key not in _prog_cache:
        _prog_cache[key] = _build_program(cls, uidseq, n_uniq)
    nc = _prog_cache[key]

    Wq = np.asarray(Wq, dtype=np.float32)
    Wk = np.asarray(Wk, dtype=np.float32)
    Wv = np.asarray(Wv, dtype=np.float32)
    Wo = np.asarray(Wo, dtype=np.float32)

    in_maps = []
    for c in range(NCORES):
        b, g = c // CPG, c % CPG
        jsl = slice(g * JC, (g + 1) * JC)
        m = {
            "hsT": np.ascontiguousarray(hsT[:, b * S:(b + 1) * S]),
            "wq": _sb_w(Wq[:, jsl]),
            "wk": _sb_w(Wk[:, jsl]),
            "wv": _sb_w(Wv[:, jsl]),
            "wo": np.ascontiguousarray(
                Wo[jsl, :].reshape(HPC, P, D).transpose(1, 0, 2)
                .reshape(P, HPC * D)).astype(np.float16),
            "cosT": cosT[b],
            "sinT": sinT[b],
            "emU": em_u[b],
        }
        in_maps.append(m)

    if os.environ.get("KERNEL_SIM"):
        from concourse.bass_interp import CoreSim
        outs = []
        for c in range(int(os.environ.get("KERNEL_SIM_CORES", "1"))):
            sim = CoreSim(nc, require_finite=False, require_nnan=True)
            for k, v in in_maps[c].items():
                sim.tensor(k)[:] = v
            sim.simulate(check_with_hw=False)
            outs.append(np.array(sim.tensor("out")).astype(np.float32))
        kernel.last_sim_partials = outs
        total = np.zeros((B, S, D), dtype=np.float32)
        for c, o in enumerate(outs):
            total[c // CPG] += o
        return total

    from concourse.bass_utils import run_bass_kernel_spmd
    trace = bool(os.environ.get("KERNEL_TRACE"))
    res = run_bass_kernel_spmd(
        nc, in_maps, core_ids=list(range(NCORES)), trace=trace)
    if trace and res.exec_time_ns is not None:
        print(f"HW exec time: {res.exec_time_ns} ns")
        kernel.last_exec_time_ns = res.exec_time_ns
        kernel.last_trace = res.instructions_and_trace
    total = np.zeros((B, S, D), dtype=np.float32)
    for c, r in enumerate(res.results):
        total[c // CPG] += np.asarray(r["out"], dtype=np.float32)
    return total
